# revision 17
# baseline (speedup 1.0000x reference)
"""Trainium2 Bass kernel for nn_Encoder_78889959293176 (Autoformer-style encoder).

Data-parallel over batch (16 batches -> 8 cores x 2). v2 layout:
  - host-folded weights: G = Wq@Wk^T (corr = x^T G x), Wvo = Wv@Wo
    (rolls commute with channel mixing), removing the q/k/v projection
    passes entirely.
  - correlation statistic via x_A^T (Gx) tiles + 2-copy diagonal shear
    through DRAM + ones-matmul reduction (unchanged mechanism).
  - AllReduce split in two (one per local batch) so the first hides
    under the second batch's correlation; on-device top-22 + masked
    softmax -> circulant band buffer (broadcast DMA trick).
  - conv1/conv2 fused per 512-col window with a 2-slot h1 ring
    (SBUF), conv weights streamed from HBM in host-pretransposed
    per-partition-contiguous layout.
  - both batches interleaved in emission order so DVE phases
    (decomposition, layernorm) hide under the other batch's matmuls.
"""

import numpy as np

import concourse.bass as bass
import concourse.bacc as bacc
import concourse.mybir as mybir
import concourse.tile as tile
from concourse import bass_utils
from concourse.alu_op_type import AluOpType

try:
    import ml_dtypes

    BF16_NP = ml_dtypes.bfloat16
except Exception:  # pragma: no cover
    BF16_NP = np.float32

F32 = mybir.dt.float32
BF16 = mybir.dt.bfloat16
AF = mybir.ActivationFunctionType

B, L, D = 16, 2048, 512
CF = 2048
TOPK = 22
KER = 25
EPS = 1e-5
SLOPE = 0.01
NCORES = 8
BPC = B // NCORES
DC = D // 128  # 4
CFC = CF // 128  # 16
TW = L // 512  # 4
TM = L // 128  # 16
NEG = -1.0e30

import os as _os_env


def _kp(name):
    return _os_env.environ.get(name, "1") == "1"


def build(nc: bass.Bass, n_group: int, lite: bool = False):
    x_dm = nc.dram_tensor("x_dm", [BPC, D, L], F32, kind="ExternalInput")
    xh_dm = nc.dram_tensor("xh_bf", [BPC, 128, DC, L], BF16, kind="ExternalInput")
    gt_d = nc.dram_tensor("gt_h", [128, DC, D], BF16, kind="ExternalInput")
    wvo_d = nc.dram_tensor("wvo_h", [128, DC, D], BF16, kind="ExternalInput")
    bop_d = nc.dram_tensor("bop_t", [128, DC], F32, kind="ExternalInput")
    w1_d = nc.dram_tensor("w1s", [CFC, 128, DC * 3, 128], BF16, kind="ExternalInput")
    w2_d = nc.dram_tensor("w2s", [DC, 128, 2, CFC * 3 // 2, 128], BF16,
                          kind="ExternalInput")
    lng_d = nc.dram_tensor("lng_t", [128, DC], F32, kind="ExternalInput")
    lnb_d = nc.dram_tensor("lnb_t", [128, DC], F32, kind="ExternalInput")
    out_dm = nc.dram_tensor("out_dm", [BPC, D, L], BF16, kind="ExternalOutput")

    with tile.TileContext(nc) as tc:
        _body(nc, tc, n_group, x_dm, xh_dm, gt_d, wvo_d, bop_d, w1_d, w2_d,
              lng_d, lnb_d, out_dm)
    return nc


def _decompose(nc, scan_pool, src, dst):
    for dci in range(DC):
        _decompose_dci(nc, scan_pool, src, dst, dci)


def _decompose_dci(nc, scan_pool, src, dst, dci):
    """dst[:, dci, 1:L+1] = src[:, dci] - movavg_KER; replicated edge cols."""
    half = (KER - 1) // 2
    if True:
        eng = nc.vector
        pad = scan_pool.tile([128, L + KER], F32, tag="scan_pad",
                             name="scan_pad")  # noqa
        cs = scan_pool.tile([128, L + KER], F32, tag="scan_cs",
                            name="scan_cs")
        eng.memset(pad[:, 0:1], 0.0)
        eng.tensor_copy(
            out=pad[:, 1 : 1 + half],
            in_=src[:, dci, 0:1].to_broadcast([128, half]),
        )
        nc.scalar.activation(pad[:, 1 + half : 1 + half + L], src[:, dci, :], AF.Copy)
        eng.tensor_copy(
            out=pad[:, 1 + half + L :],
            in_=src[:, dci, L - 1 : L].to_broadcast([128, half]),
        )
        eng.tensor_tensor_scan(
            out=cs[:], data0=pad[:], data1=pad[:], initial=0.0,
            op0=AluOpType.add, op1=AluOpType.bypass,
        )
        # d1 reuses pad (dead after the scan)
        eng.tensor_sub(out=pad[:, 0:L], in0=cs[:, KER:], in1=cs[:, 0:L])
        eng.scalar_tensor_tensor(
            out=dst[:, dci, 1 : L + 1], in0=pad[:, 0:L], scalar=-1.0 / KER,
            in1=src[:, dci, :], op0=AluOpType.mult, op1=AluOpType.add,
        )
        eng.tensor_copy(out=dst[:, dci, 0:1], in_=dst[:, dci, 1:2])
        eng.tensor_copy(
            out=dst[:, dci, L + 1 : L + 2], in_=dst[:, dci, L : L + 1]
        )


def _body(nc, tc, n_group, x_dm, xh_dm, gt_d, wvo_d, bop_d, w1_d, w2_d,
          lng_d, lnb_d, out_dm):
    with (
        tc.tile_pool(name="p0", bufs=1) as p0,
        tc.tile_pool(name="pp", bufs=4, space="PSUM") as pp,
        tc.tile_pool(name="dr", bufs=1, space="DRAM") as dr,
        tc.tile_pool(name="dr3", bufs=4, space="DRAM") as dr3,
        tc.tile_pool(name="pseab", bufs=2) as pseab,
        tc.tile_pool(name="pysb", bufs=1) as pysb,
        tc.tile_pool(name="psea2", bufs=1) as psea2,
        tc.tile_pool(name="pscan", bufs=1) as pscan,
    ):
        # ---------------- persistent constants ----------------
        ones_bf = p0.tile([128, 1], BF16, tag="ones_bf")
        nc.vector.memset(ones_bf[:], 1.0 / D)
        bop_c = p0.tile([128, DC], F32, tag="bop_c")
        lng_c = p0.tile([128, DC], F32, tag="lng_c")
        lnb_c = p0.tile([128, DC], F32, tag="lnb_c")
        nc.sync.dma_start(bop_c[:], bop_d[:, :])
        nc.sync.dma_start(lng_c[:], lng_d[:, :])
        nc.sync.dma_start(lnb_c[:], lnb_d[:, :])
        hb = []
        seab = []

        with tc.tile_pool(name="pxw", bufs=2) as pxw:
            xw = []
            gt_s = pxw.tile([128, DC, D], BF16, tag="gt_s")
            wvo_s = pxw.tile([128, DC, D], BF16, tag="wvo_s")
            nc.sync.dma_start(gt_s[:], gt_d[:, :, :])
            nc.sync.dma_start(wvo_s[:], wvo_d[:, :, :])
            pgbuf_ctx = tc.tile_pool(name="pgbuf", bufs=2)
            pgbuf = pgbuf_ctx.__enter__()
            gbufs = []
            pmv_ctx = tc.tile_pool(name="pmv", bufs=1)
            pmv = pmv_ctx.__enter__()
            mvf = pmv.tile([1, BPC * L], F32, tag="mvf")
            cco = []
            # ============ phase 1: mean_value + per-batch AllReduce ========
            with (
                tc.tile_pool(name="pxb", bufs=1) as pxb,
                tc.tile_pool(name="pxg", bufs=1) as pxg,
                tc.tile_pool(name="pwa", bufs=2) as pwa,
                tc.tile_pool(name="pcsb", bufs=2) as pcsb,
                tc.tile_pool(name="ppm1", bufs=1, space="PSUM") as ppm1,
            ):
                xbs = []
                for b in range(BPC):
                    xb = pxb.tile([128, DC, L], BF16, tag="xb")
                    if b == 0:
                        for w in range(TW):
                            nc.sync.dma_start(
                                xb[:, :, 512 * w : 512 * w + 512],
                                xh_dm.ap()[b, :, :, 512 * w : 512 * w + 512],
                            )
                    else:
                        nc.gpsimd.dma_start(xb[:], xh_dm.ap()[b])
                    xbs.append(xb)

                    # xg = (Wq Wk^T) x  (d-major)
                    xg = pxg.tile([128, DC, L], BF16, tag="xg")
                    for dco in range(DC):
                        for twi in range(TW):
                            ps = pp.tile([128, 512], F32, tag="ps")
                            for dci in range(DC):
                                nc.tensor.matmul(
                                    ps[:],
                                    lhsT=gt_s[:, dci, 128 * dco : 128 * dco + 128],
                                    rhs=xb[:, dci, 512 * twi : 512 * twi + 512],
                                    start=(dci == 0), stop=(dci == DC - 1),
                                )
                            if (dco + twi) % 2 == 0:
                                nc.scalar.activation(
                                    xg[:, dco, 512 * twi : 512 * twi + 512],
                                    ps[:], AF.Copy,
                                )
                            else:
                                nc.vector.tensor_copy(
                                    out=xg[:, dco, 512 * twi : 512 * twi + 512],
                                    in_=ps[:],
                                )

                    # corr tiles + diagonal shear + ones-matmul lag reduction
                    mv_reg = [
                        ppm1.tile([1, 512], F32, tag=f"mv{cc}", name=f"mv_{cc}")
                        for cc in range(4)
                    ]

                    def _emit_mv(A, wa, mv_reg=mv_reg):
                        for cc in range(4):
                            w0 = (512 * cc + 128 * A) % L
                            nc.tensor.matmul(
                                mv_reg[cc][0:1, :],
                                lhsT=ones_bf[:],
                                rhs=wa[:, w0 : w0 + 512],
                                start=(A == 0), stop=(A == TM - 1),
                            )

                    def _xwproj(xb=xb):
                        xw_b = pxw.tile([128, TM, D], BF16, tag="xw",
                                        name="xw_b")
                        for tm in range(TM if _kp("KP3") else 0):
                            ps = pp.tile([128, 512], F32, tag="ps")
                            for dci in range(DC):
                                nc.tensor.matmul(
                                    ps[:],
                                    lhsT=xb[:, dci, 128 * tm : 128 * tm + 128],
                                    rhs=wvo_s[:, dci, :],
                                    start=(dci == 0), stop=(dci == DC - 1),
                                )
                            if tm % 2 == 0:
                                nc.scalar.activation(xw_b[:, tm, :], ps[:], AF.Copy)
                            else:
                                nc.vector.tensor_copy(out=xw_b[:, tm, :], in_=ps[:])
                        xw.append(xw_b)

                    pend = []
                    for A in range(TM if _kp("KP1") else 0):
                        if A == 8:
                            _xwproj()  # frees the xb slot before the next
                            # batch's load; fills PE during the shear tail
                        bufA = dr3.tile([128, 4224], BF16, tag="bufA")
                        for tB in range(TW):
                            psc = pp.tile([128, 512], F32, tag="ps")
                            for dci in range(DC):
                                nc.tensor.matmul(
                                    psc[:],
                                    lhsT=xb[:, dci, 128 * A : 128 * A + 128],
                                    rhs=xg[:, dci, 512 * tB : 512 * tB + 512],
                                    start=(dci == 0), stop=(dci == DC - 1),
                                )
                            c_sb = pcsb.tile([128, 512], BF16, tag="c_sb")
                            if tB % 2 == 0:
                                nc.scalar.activation(c_sb[:], psc[:], AF.Copy)
                            else:
                                nc.vector.tensor_copy(out=c_sb[:], in_=psc[:])
                            for cp, eng in ((0, nc.sync), (1, nc.scalar)):
                                dst = bass.AP(
                                    bufA[:].tensor,
                                    127 + 512 * tB + 2048 * cp,
                                    [[4223, 128], [1, 512]],
                                )
                                eng.dma_start(dst, c_sb[:])
                        wa = pwa.tile([128, 2560], BF16, tag="wa")
                        nc.sync.dma_start(
                            wa[:],
                            bass.AP(bufA[:].tensor, 128, [[4224, 128], [1, 2560]]),
                        )
                        pend.append((A, wa))
                        if len(pend) > 1:
                            _emit_mv(*pend.pop(0))
                    for a_w in pend:
                        _emit_mv(*a_w)
                    for cc in range(4):
                        nc.scalar.activation(
                            mvf[0:1, L * b + 512 * cc : L * b + 512 * cc + 512],
                            mv_reg[cc][0:1, :], AF.Copy,
                        )

                    # per-batch AllReduce, issued as soon as this mv is done
                    cci_b = dr.tile([1, L], F32, tag=f"cci{b}")
                    cco_b = dr.tile([1, L], F32, tag=f"cco{b}")
                    nc.gpsimd.dma_start(cci_b[:], mvf[0:1, L * b : L * b + L])
                    _selfcc = _os_env.environ.get("KERNEL_SELFCC", "0") == "1"
                    _nocc = _os_env.environ.get("KERNEL_NOCC", "0") == "1"
                    if _nocc:
                        nc.gpsimd.dma_start(cco_b[:], cci_b[:])
                    else:
                        nc.gpsimd.collective_compute(
                            "AllReduce", AluOpType.add,
                            replica_groups=(
                                [[c] for c in range(n_group)] if _selfcc
                                else [list(range(n_group))]
                            ),
                            ins=[cci_b[:].opt()], outs=[cco_b[:].opt()],
                        )
                    cco.append(cco_b)
                    if not _kp("KP1"):
                        _xwproj()

            # ---- phase 2: combine ARs, topk, masked softmax, band bufs
            with (
                tc.tile_pool(name="p12", bufs=1) as p12,
                tc.tile_pool(name="ppw", bufs=1, space="PSUM") as ppw,
            ):
                bsum = p12.tile([1, L], F32, tag="bsum")
                work = p12.tile([1, L], F32, tag="work")
                mask = p12.tile([1, L], F32, tag="mask")
                nbias = p12.tile([1, 1], F32, tag="nbias")
                nc.vector.memset(nbias[:], -1.0e4)
                pwm = ppw.tile([128, 512], F32, tag="pwm")

                def _warm(t):
                    # dummy matmul keyed on a just-written [1,*] tile: keeps
                    # the PE HAM window busy through the dependency-bound gap
                    nc.tensor.matmul(
                        pwm[:], lhsT=t[0:1, 0:128], rhs=t[0:1, 0:512],
                        start=True, stop=True,
                    )

                nc.gpsimd.dma_start(bsum[:], cco[0][:])
                nc.gpsimd.dma_start(work[:], cco[1][:])
                nc.vector.tensor_add(out=bsum[:], in0=bsum[:], in1=work[:])
                _warm(bsum)

                t_on = bsum
                for r, kk in enumerate((8, 8, TOPK - 16)):
                    mx8 = p12.tile([1, 8], F32, tag=f"mx8_{r}")
                    nc.vector.max(out=mx8[:], in_=t_on[:])
                    if kk < 8:
                        nc.vector.memset(mx8[:, kk:8], NEG)
                    nc.vector.match_replace(
                        out=work[:], in_to_replace=mx8[:], in_values=t_on[:],
                        imm_value=NEG,
                    )
                    t_on = work
                    _warm(work)
                nc.vector.tensor_sub(out=mask[:], in0=bsum[:], in1=work[:])
                nc.vector.tensor_scalar_min(mask[:], mask[:], 1.0)
                _warm(mask)

                for b in range(BPC):
                    # softmax over the 22 kept lags: gf = (mv+1e4)*mask,
                    # exp(gf-1e4) -> masked-out lanes underflow to exact 0
                    gf = bsum  # bsum is dead after the mask; reuse its slot
                    nc.vector.scalar_tensor_tensor(
                        out=gf[:], in0=mvf[0:1, L * b : L * b + L],
                        scalar=1.0e4, in1=mask[:],
                        op0=AluOpType.add, op1=AluOpType.mult,
                    )
                    nc.scalar.activation(gf[:], gf[:], AF.Exp, bias=nbias[0:1, 0:1])
                    _warm(gf)
                    zz = p12.tile([1, 1], F32, tag="sm_z")
                    nc.vector.reduce_sum(
                        out=zz[:], in_=gf[:], axis=mybir.AxisListType.X
                    )
                    nc.vector.reciprocal(out=zz[:], in_=zz[:])
                    gfb = p12.tile([1, L], BF16, tag=f"gfb{b}")
                    nc.vector.tensor_scalar_mul(gfb[:], gf[:], zz[:])
                    # periodic replication into DRAM; a row-step-2047 read
                    # yields the circulant band gbuf[p,m] = g[(127-p+m)%L].
                    # write and read are split across the sync+scalar rings
                    # to halve the serial DMA latency on the critical path.
                    hbuf = dr.tile([1, 129 * L], BF16, tag=f"hb{b}")
                    _gs = gfb[:]
                    _ga = [list(p) for p in _gs.ap]
                    hview = hbuf[:].rearrange("a (r n) -> a r n", r=129)
                    nc.sync.dma_start(
                        hview[:, 0:65, :],
                        bass.AP(_gs.tensor, _gs.offset,
                                [_ga[0], [0, 65], _ga[-1]]),
                    )
                    nc.scalar.dma_start(
                        hview[:, 65:129, :],
                        bass.AP(_gs.tensor, _gs.offset,
                                [_ga[0], [0, 64], _ga[-1]]),
                    )
                    hb.append(hbuf)
                    gbuf = pgbuf.tile([128, 3968], BF16, tag="gbuf",
                                      name="gbuf")
                    nc.sync.dma_start(
                        gbuf[0:64, :],
                        bass.AP(hbuf[:].tensor, 127, [[2047, 64], [1, 3968]]),
                    )
                    nc.scalar.dma_start(
                        gbuf[64:128, :],
                        bass.AP(hbuf[:].tensor, 127 + 2047 * 64,
                                [[2047, 64], [1, 3968]]),
                    )
                    gbufs.append(gbuf)

            pmv_ctx.__exit__(None, None, None)
            # ============ phase 3+4 per batch: circulant + decomp ==========
            with (
                tc.tile_pool(name="pacx", bufs=2) as pacx,
                tc.tile_pool(name="pxr", bufs=3) as pxr,
            ):
                for b in range(BPC):
                    gbuf = gbufs[b]
                    acx = pacx.tile([128, DC, L], BF16, tag="acx")
                    for dm in range(DC if _kp("KP3") else 0):
                        for nw in range(TW):
                            ps = pp.tile([128, 512], F32, tag="ps")
                            for Bc in range(TM):
                                gp = 512 * nw - 128 * Bc + 1920
                                nc.tensor.matmul(
                                    ps[:],
                                    lhsT=xw[b][:, Bc, 128 * dm : 128 * dm + 128],
                                    rhs=gbuf[:, gp : gp + 512],
                                    start=(Bc == 0), stop=(Bc == TM - 1),
                                )
                            xr = pxr.tile([128, 512], F32, tag="xr")
                            nc.sync.dma_start(
                                xr[:],
                                x_dm.ap()[b, 128 * dm : 128 * dm + 128,
                                          512 * nw : 512 * nw + 512],
                            )
                            nc.vector.scalar_tensor_tensor(
                                out=acx[:, dm, 512 * nw : 512 * nw + 512],
                                in0=ps[:], scalar=bop_c[:, dm : dm + 1], in1=xr[:],
                                op0=AluOpType.add, op1=AluOpType.add,
                            )
                    sb = pseab.tile([128, DC, L + 2], BF16, tag="seab")
                    if _kp("KP4"):
                        _decompose(nc, pscan, acx, sb)
                    seab.append(sb)

            pgbuf_ctx.__exit__(None, None, None)

        # ============ conv + decomp2 + layernorm, interleaved =============
        with (
            tc.tile_pool(name="pw1", bufs=4) as pw1,
            tc.tile_pool(name="pw2", bufs=2) as pw2,
            tc.tile_pool(name="ph1r", bufs=1) as ph1r,
            tc.tile_pool(name="pc5", bufs=2) as pc5,
            tc.tile_pool(name="pln", bufs=1) as pln,
            tc.tile_pool(name="pog", bufs=2) as pog,
            tc.tile_pool(name="ppm7", bufs=2, space="PSUM") as ppm7,
        ):
            # h1 ring: 2 self-contained window slots [left halo | 512 | right]
            h1s = [
                ph1r.tile([128, CFC, 514], BF16, tag=f"h1s{s}", name=f"h1s_{s}")
                for s in range(2)
            ]

            def conv1_win(b, nw):
                slot = h1s[nw % 2]
                sb = seab[b]
                for co in range(CFC if _kp("KP5") else 0):
                    w1t = pw1.tile([128, DC * 3, 128], BF16, tag="w1t")
                    nc.sync.dma_start(w1t[:], w1_d.ap()[co])
                    ps = pp.tile([128, 512], F32, tag="ps")
                    first = True
                    for dci in range(DC):
                        for tap in range(3):
                            nc.tensor.matmul(
                                ps[:],
                                lhsT=w1t[:, 3 * dci + tap, :],
                                rhs=sb[:, dci,
                                       512 * nw + tap : 512 * nw + tap + 512],
                                start=first, stop=(dci == DC - 1 and tap == 2),
                            )
                            first = False
                    nc.scalar.activation(
                        slot[:, co, 1:513], ps[:], AF.Lrelu, alpha=SLOPE
                    )
                # halo columns
                if nw == 0:
                    nc.vector.tensor_copy(out=slot[:, :, 0:1], in_=slot[:, :, 1:2])
                else:
                    nc.vector.tensor_copy(
                        out=slot[:, :, 0:1], in_=h1s[(nw - 1) % 2][:, :, 512:513]
                    )
                    nc.vector.tensor_copy(
                        out=h1s[(nw - 1) % 2][:, :, 513:514], in_=slot[:, :, 1:2]
                    )
                if nw == TW - 1:
                    nc.vector.tensor_copy(
                        out=slot[:, :, 513:514], in_=slot[:, :, 512:513]
                    )

            def conv2_win(b, nw, ysb, post_co=None):
                slot = h1s[nw % 2]
                for co in range(DC if _kp("KP6") else 0):
                    ps = pp.tile([128, 512], F32, tag="ps")
                    first = True
                    for hw in range(2):
                        w2t = pw2.tile([128, CFC * 3 // 2, 128], BF16, tag="w2t")
                        nc.scalar.dma_start(w2t[:], w2_d.ap()[co, :, hw])
                        for k in range(CFC * 3 // 2):
                            ci, tap = divmod(hw * CFC * 3 // 2 + k, 3)
                            nc.tensor.matmul(
                                ps[:],
                                lhsT=w2t[:, k, :],
                                rhs=slot[:, ci, tap : tap + 512],
                                start=first,
                                stop=(hw == 1 and k == CFC * 3 // 2 - 1),
                            )
                            first = False
                    h2r = pc5.tile([128, 512], F32, tag="h2r")
                    nc.scalar.activation(h2r[:], ps[:], AF.Lrelu, alpha=SLOPE)
                    nc.vector.tensor_add(
                        out=ysb[:, co, 512 * nw : 512 * nw + 512],
                        in0=h2r[:],
                        in1=seab[b][:, co, 1 + 512 * nw : 513 + 512 * nw],
                    )
                    if post_co is not None:
                        post_co(co)

            def conv_batch(b, ysb, post_co=None):
                conv1_win(b, 0)
                conv1_win(b, 1)
                conv2_win(b, 0, ysb)
                conv1_win(b, 2)
                conv2_win(b, 1, ysb)
                conv1_win(b, 3)
                conv2_win(b, 2, ysb)
                conv2_win(b, 3, ysb, post_co=post_co)

            def phase7(b, ysb, sea2):
                stats = pln.tile([1, 2 * L], F32, tag="stats")
                for twi in range(TW if _kp("KP7") else 0):
                    st_s = ppm7.tile([1, 512], F32, tag="st_s")
                    st_q = ppm7.tile([1, 512], F32, tag="st_q")
                    for dci in range(DC):
                        sqt = pc5.tile([128, 512], BF16, tag="sqt")
                        nc.scalar.activation(
                            sqt[:],
                            sea2[:, dci, 1 + 512 * twi : 513 + 512 * twi],
                            AF.Square,
                        )
                        nc.tensor.matmul(
                            st_s[0:1, :], lhsT=ones_bf[:],
                            rhs=sea2[:, dci, 1 + 512 * twi : 513 + 512 * twi],
                            start=(dci == 0), stop=(dci == DC - 1),
                        )
                        nc.tensor.matmul(
                            st_q[0:1, :], lhsT=ones_bf[:], rhs=sqt[:],
                            start=(dci == 0), stop=(dci == DC - 1),
                        )
                    nc.scalar.activation(
                        stats[0:1, 512 * twi : 512 * twi + 512],
                        st_s[0:1, :], AF.Copy,
                    )
                    nc.scalar.activation(
                        stats[0:1, L + 512 * twi : L + 512 * twi + 512],
                        st_q[0:1, :], AF.Copy,
                    )
                if _kp("KP7"):
                    # fold stats [1,2L] -> [128,16]x2 via DRAM for fast rsqrt
                    st_d = dr.tile([1, 2 * L], F32, tag=f"st_d{b}")
                    nc.sync.dma_start(st_d[:], stats[:])
                    muf = pln.tile([128, 16], F32, tag="muf")
                    msf = pln.tile([128, 16], F32, tag="msf")
                    nc.sync.dma_start(
                        muf[:], bass.AP(st_d[:].tensor, 0, [[16, 128], [1, 16]])
                    )
                    nc.sync.dma_start(
                        msf[:], bass.AP(st_d[:].tensor, L, [[16, 128], [1, 16]])
                    )
                    varf = pln.tile([128, 16], F32, tag="varf")
                    nc.vector.tensor_mul(out=varf[:], in0=muf[:], in1=muf[:])
                    nc.vector.tensor_sub(out=varf[:], in0=msf[:], in1=varf[:])
                    nc.vector.tensor_scalar_add(varf[:], varf[:], EPS)
                    nc.vector.reciprocal(out=varf[:], in_=varf[:])
                    nc.scalar.activation(varf[:], varf[:], AF.Sqrt)
                    rs_d = dr.tile([1, L], F32, tag=f"rs_d{b}")
                    nc.sync.dma_start(
                        bass.AP(rs_d[:].tensor, 0, [[16, 128], [1, 16]]), varf[:]
                    )
                    mub = pln.tile([128, L], F32, tag="mub")
                    rsb = pln.tile([128, L], F32, tag="rsb")
                    nc.sync.dma_start(
                        mub[:], bass.AP(st_d[:].tensor, 0, [[0, 128], [1, L]])
                    )
                    nc.sync.dma_start(
                        rsb[:], bass.AP(rs_d[:].tensor, 0, [[0, 128], [1, L]])
                    )
                for dci in range(DC if _kp("KP7") else 0):
                    eng = nc.vector
                    og = pog.tile([128, L], BF16, tag="og")
                    eng.tensor_sub(
                        out=og[:], in0=sea2[:, dci, 1 : L + 1], in1=mub[:]
                    )
                    eng.tensor_mul(out=og[:], in0=og[:], in1=rsb[:])
                    nc.scalar.activation(
                        og[:], og[:], AF.Identity,
                        bias=lnb_c[:, dci : dci + 1], scale=lng_c[:, dci : dci + 1],
                    )
                    nc.scalar.dma_start(
                        out_dm.ap()[b, 128 * dci : 128 * dci + 128, :], og[:]
                    )

            ysb0 = pysb.tile([128, DC, L], BF16, tag="ysb")
            sea2_0 = psea2.tile([128, DC, L + 2], BF16, tag="sea2",
                                name="sea2_0")
            conv_batch(0, ysb0, post_co=(
                (lambda co: _decompose_dci(nc, pscan, ysb0, sea2_0, co))
                if _kp("KP7") else None))
            phase7(0, ysb0, sea2_0)
            ysb1 = pysb.tile([128, DC, L], BF16, tag="ysb")
            sea2_1 = psea2.tile([128, DC, L + 2], BF16, tag="sea2",
                                name="sea2_1")
            conv_batch(1, ysb1, post_co=(
                (lambda co: _decompose_dci(nc, pscan, ysb1, sea2_1, co))
                if _kp("KP7") else None))
            phase7(1, ysb1, sea2_1)


# ---------------------------------------------------------------------------
# host side
# ---------------------------------------------------------------------------
_CACHE = {}


def _get_nc(n_group: int):
    if n_group not in _CACHE:
        nc = bacc.Bacc("TRN2", target_bir_lowering=False, debug=False,
                       num_devices=n_group)
        build(nc, n_group)
        nc.compile()
        _CACHE[n_group] = nc
    return _CACHE[n_group]


def stage_inputs(inputs, ncores=NCORES):
    x = np.asarray(inputs["x"], np.float32)
    Wq = np.asarray(inputs["Wq"], np.float32)
    Wk = np.asarray(inputs["Wk"], np.float32)
    Wv = np.asarray(inputs["Wv"], np.float32)
    Wo = np.asarray(inputs["Wo"], np.float32)
    bv = np.asarray(inputs["bv"], np.float32)
    bo = np.asarray(inputs["bo"], np.float32)
    w1 = np.asarray(inputs["conv1_w"], np.float32)
    w2 = np.asarray(inputs["conv2_w"], np.float32)
    lng = np.asarray(inputs["ln_g"], np.float32)
    lnb = np.asarray(inputs["ln_b"], np.float32)

    bop = bo + bv @ Wo
    col = lambda v: np.ascontiguousarray(v.reshape(DC, 128).T)
    dmaj = lambda M: np.ascontiguousarray(
        M.reshape(DC, 128, D).transpose(1, 0, 2)
    ).astype(BF16_NP)
    # corr = x^T (Wq Wk^T) x  ->  xg = (Wq Wk^T) x, staged pre-transposed
    gt_h = dmaj(Wk @ Wq.T)
    # rolls commute with channel mixing: fold Wv@Wo
    wvo_h = dmaj(Wv @ Wo)
    w1s = np.ascontiguousarray(
        w1.reshape(3, DC, 128, CFC, 128).transpose(3, 2, 1, 0, 4)
    ).reshape(CFC, 128, DC * 3, 128).astype(BF16_NP)
    # w2 staged as [co, p, 48, 128] with the 48 (ci,tap) pairs in order,
    # then split into two halves of 24 for streaming
    w2s = np.ascontiguousarray(
        w2.reshape(3, CFC, 128, DC, 128).transpose(3, 2, 1, 0, 4)
    ).reshape(DC, 128, 2, CFC * 3 // 2, 128).astype(BF16_NP)

    shared = {
        "gt_h": gt_h, "wvo_h": wvo_h, "bop_t": col(bop),
        "w1s": w1s, "w2s": w2s, "lng_t": col(lng), "lnb_t": col(lnb),
    }
    bpc = B // ncores
    in_maps = []
    for c in range(ncores):
        m = dict(shared)
        xc = np.ascontiguousarray(x[bpc * c : bpc * (c + 1)].transpose(0, 2, 1))
        m["x_dm"] = xc
        m["xh_bf"] = np.ascontiguousarray(
            xc.reshape(bpc, DC, 128, L).transpose(0, 2, 1, 3)
        ).astype(BF16_NP)
        in_maps.append(m)
    return in_maps


def kernel(**inputs):
    nc = _get_nc(NCORES)
    in_maps = stage_inputs(inputs)
    res = bass_utils.run_bass_kernel_spmd(nc, in_maps, core_ids=list(range(NCORES)))
    out = np.empty((B, L, D), np.float32)
    for c in range(NCORES):
        o = np.asarray(res.results[c]["out_dm"])  # [BPC, D, L] bf16
        for i in range(BPC):
            out[BPC * c + i] = o[i].T.astype(np.float32)
    return out


# revision 18
# speedup vs baseline: 1.0247x; 1.0247x over previous
"""Trainium2 Bass kernel for nn_Encoder_78889959293176 (Autoformer-style encoder).

Data-parallel over batch (16 batches -> 8 cores x 2). v2 layout:
  - host-folded weights: G = Wq@Wk^T (corr = x^T G x), Wvo = Wv@Wo
    (rolls commute with channel mixing), removing the q/k/v projection
    passes entirely.
  - correlation statistic via x_A^T (Gx) tiles + 2-copy diagonal shear
    through DRAM + ones-matmul reduction (unchanged mechanism).
  - AllReduce split in two (one per local batch) so the first hides
    under the second batch's correlation; on-device top-22 + masked
    softmax -> circulant band buffer (broadcast DMA trick).
  - conv1/conv2 fused per 512-col window with a 2-slot h1 ring
    (SBUF), conv weights streamed from HBM in host-pretransposed
    per-partition-contiguous layout.
  - both batches interleaved in emission order so DVE phases
    (decomposition, layernorm) hide under the other batch's matmuls.
"""

import numpy as np

import concourse.bass as bass
import concourse.bacc as bacc
import concourse.mybir as mybir
import concourse.tile as tile
from concourse import bass_utils
from concourse.alu_op_type import AluOpType

try:
    import ml_dtypes

    BF16_NP = ml_dtypes.bfloat16
except Exception:  # pragma: no cover
    BF16_NP = np.float32

F32 = mybir.dt.float32
BF16 = mybir.dt.bfloat16
AF = mybir.ActivationFunctionType

B, L, D = 16, 2048, 512
CF = 2048
TOPK = 22
KER = 25
EPS = 1e-5
SLOPE = 0.01
NCORES = 8
BPC = B // NCORES
DC = D // 128  # 4
CFC = CF // 128  # 16
TW = L // 512  # 4
TM = L // 128  # 16
NEG = -1.0e30

import os as _os_env


def _kp(name):
    return _os_env.environ.get(name, "1") == "1"


def build(nc: bass.Bass, n_group: int, lite: bool = False):
    x_dm = nc.dram_tensor("x_dm", [BPC, D, L], F32, kind="ExternalInput")
    xh_dm = nc.dram_tensor("xh_bf", [BPC, 128, DC, L], BF16, kind="ExternalInput")
    gt_d = nc.dram_tensor("gt_h", [128, DC, D], BF16, kind="ExternalInput")
    wvo_d = nc.dram_tensor("wvo_h", [128, DC, D], BF16, kind="ExternalInput")
    bop_d = nc.dram_tensor("bop_t", [128, DC], F32, kind="ExternalInput")
    w1_d = nc.dram_tensor("w1s", [CFC, 128, DC * 3, 128], BF16, kind="ExternalInput")
    w2_d = nc.dram_tensor("w2s", [DC, 128, 2, CFC * 3 // 2, 128], BF16,
                          kind="ExternalInput")
    lng_d = nc.dram_tensor("lng_t", [128, DC], F32, kind="ExternalInput")
    lnb_d = nc.dram_tensor("lnb_t", [128, DC], F32, kind="ExternalInput")
    out_dm = nc.dram_tensor("out_dm", [BPC, D, L], BF16, kind="ExternalOutput")

    with tile.TileContext(nc) as tc:
        _body(nc, tc, n_group, x_dm, xh_dm, gt_d, wvo_d, bop_d, w1_d, w2_d,
              lng_d, lnb_d, out_dm)
    return nc


def _decompose(nc, scan_pool, src, dst):
    for dci in range(DC):
        _decompose_dci(nc, scan_pool, src, dst, dci)


def _decompose_dci(nc, scan_pool, src, dst, dci):
    """dst[:, dci, 1:L+1] = src[:, dci] - movavg_KER; replicated edge cols."""
    half = (KER - 1) // 2
    if True:
        eng = nc.vector
        pad = scan_pool.tile([128, L + KER], F32, tag="scan_pad",
                             name="scan_pad")  # noqa
        cs = scan_pool.tile([128, L + KER], F32, tag="scan_cs",
                            name="scan_cs")
        eng.memset(pad[:, 0:1], 0.0)
        eng.tensor_copy(
            out=pad[:, 1 : 1 + half],
            in_=src[:, dci, 0:1].to_broadcast([128, half]),
        )
        nc.scalar.activation(pad[:, 1 + half : 1 + half + L], src[:, dci, :], AF.Copy)
        eng.tensor_copy(
            out=pad[:, 1 + half + L :],
            in_=src[:, dci, L - 1 : L].to_broadcast([128, half]),
        )
        eng.tensor_tensor_scan(
            out=cs[:], data0=pad[:], data1=pad[:], initial=0.0,
            op0=AluOpType.add, op1=AluOpType.bypass,
        )
        # d1 reuses pad (dead after the scan)
        eng.tensor_sub(out=pad[:, 0:L], in0=cs[:, KER:], in1=cs[:, 0:L])
        eng.scalar_tensor_tensor(
            out=dst[:, dci, 1 : L + 1], in0=pad[:, 0:L], scalar=-1.0 / KER,
            in1=src[:, dci, :], op0=AluOpType.mult, op1=AluOpType.add,
        )
        eng.tensor_copy(out=dst[:, dci, 0:1], in_=dst[:, dci, 1:2])
        eng.tensor_copy(
            out=dst[:, dci, L + 1 : L + 2], in_=dst[:, dci, L : L + 1]
        )


def _body(nc, tc, n_group, x_dm, xh_dm, gt_d, wvo_d, bop_d, w1_d, w2_d,
          lng_d, lnb_d, out_dm):
    with (
        tc.tile_pool(name="p0", bufs=1) as p0,
        tc.tile_pool(name="pp", bufs=4, space="PSUM") as pp,
        tc.tile_pool(name="dr", bufs=1, space="DRAM") as dr,
        tc.tile_pool(name="dr3", bufs=4, space="DRAM") as dr3,
        tc.tile_pool(name="pseab", bufs=2) as pseab,
        tc.tile_pool(name="pysb", bufs=1) as pysb,
        tc.tile_pool(name="psea2", bufs=1) as psea2,
        tc.tile_pool(name="pscan", bufs=1) as pscan,
    ):
        # ---------------- persistent constants ----------------
        ones_bf = p0.tile([128, 1], BF16, tag="ones_bf")
        nc.vector.memset(ones_bf[:], 1.0 / D)
        bop_c = p0.tile([128, DC], F32, tag="bop_c")
        lng_c = p0.tile([128, DC], F32, tag="lng_c")
        lnb_c = p0.tile([128, DC], F32, tag="lnb_c")
        nc.sync.dma_start(bop_c[:], bop_d[:, :])
        nc.sync.dma_start(lng_c[:], lng_d[:, :])
        nc.sync.dma_start(lnb_c[:], lnb_d[:, :])
        hb = []
        seab = []

        with tc.tile_pool(name="pxw", bufs=2) as pxw:
            xw = []
            gt_s = pxw.tile([128, DC, D], BF16, tag="gt_s")
            wvo_s = pxw.tile([128, DC, D], BF16, tag="wvo_s")
            nc.sync.dma_start(gt_s[:], gt_d[:, :, :])
            nc.sync.dma_start(wvo_s[:], wvo_d[:, :, :])
            pgbuf_ctx = tc.tile_pool(name="pgbuf", bufs=2)
            pgbuf = pgbuf_ctx.__enter__()
            gbufs = []
            pmv_ctx = tc.tile_pool(name="pmv", bufs=1)
            pmv = pmv_ctx.__enter__()
            mvf = pmv.tile([1, BPC * L], F32, tag="mvf")
            cco = []
            # ============ phase 1: mean_value + per-batch AllReduce ========
            with (
                tc.tile_pool(name="pxb", bufs=1) as pxb,
                tc.tile_pool(name="pxg", bufs=1) as pxg,
                tc.tile_pool(name="pwa", bufs=2) as pwa,
                tc.tile_pool(name="pcsb", bufs=2) as pcsb,
                tc.tile_pool(name="ppm1", bufs=1, space="PSUM") as ppm1,
            ):
                xbs = []
                for b in range(BPC):
                    xb = pxb.tile([128, DC, L], BF16, tag="xb")
                    if b == 0:
                        for w in range(TW):
                            nc.sync.dma_start(
                                xb[:, :, 512 * w : 512 * w + 512],
                                xh_dm.ap()[b, :, :, 512 * w : 512 * w + 512],
                            )
                    else:
                        nc.sync.dma_start(xb[:], xh_dm.ap()[b])
                    xbs.append(xb)

                    # xg = (Wq Wk^T) x  (d-major)
                    xg = pxg.tile([128, DC, L], BF16, tag="xg")
                    for dco in range(DC):
                        for twi in range(TW):
                            ps = pp.tile([128, 512], F32, tag="ps")
                            for dci in range(DC):
                                nc.tensor.matmul(
                                    ps[:],
                                    lhsT=gt_s[:, dci, 128 * dco : 128 * dco + 128],
                                    rhs=xb[:, dci, 512 * twi : 512 * twi + 512],
                                    start=(dci == 0), stop=(dci == DC - 1),
                                )
                            if (dco + twi) % 2 == 0:
                                nc.scalar.activation(
                                    xg[:, dco, 512 * twi : 512 * twi + 512],
                                    ps[:], AF.Copy,
                                )
                            else:
                                nc.vector.tensor_copy(
                                    out=xg[:, dco, 512 * twi : 512 * twi + 512],
                                    in_=ps[:],
                                )

                    # corr tiles + diagonal shear + ones-matmul lag reduction
                    mv_reg = [
                        ppm1.tile([1, 512], F32, tag=f"mv{cc}", name=f"mv_{cc}")
                        for cc in range(4)
                    ]

                    def _emit_mv(A, wa, mv_reg=mv_reg):
                        for cc in range(4):
                            w0 = (512 * cc + 128 * A) % L
                            nc.tensor.matmul(
                                mv_reg[cc][0:1, :],
                                lhsT=ones_bf[:],
                                rhs=wa[:, w0 : w0 + 512],
                                start=(A == 0), stop=(A == TM - 1),
                            )

                    def _xwproj(xb=xb):
                        xw_b = pxw.tile([128, TM, D], BF16, tag="xw",
                                        name="xw_b")
                        for tm in range(TM if _kp("KP3") else 0):
                            ps = pp.tile([128, 512], F32, tag="ps")
                            for dci in range(DC):
                                nc.tensor.matmul(
                                    ps[:],
                                    lhsT=xb[:, dci, 128 * tm : 128 * tm + 128],
                                    rhs=wvo_s[:, dci, :],
                                    start=(dci == 0), stop=(dci == DC - 1),
                                )
                            if tm % 2 == 0:
                                nc.scalar.activation(xw_b[:, tm, :], ps[:], AF.Copy)
                            else:
                                nc.vector.tensor_copy(out=xw_b[:, tm, :], in_=ps[:])
                        xw.append(xw_b)

                    pend = []
                    for A in range(TM if _kp("KP1") else 0):
                        if A == 8:
                            _xwproj()  # frees the xb slot before the next
                            # batch's load; fills PE during the shear tail
                        bufA = dr3.tile([128, 4224], BF16, tag="bufA")
                        for tB in range(TW):
                            psc = pp.tile([128, 512], F32, tag="ps")
                            for dci in range(DC):
                                nc.tensor.matmul(
                                    psc[:],
                                    lhsT=xb[:, dci, 128 * A : 128 * A + 128],
                                    rhs=xg[:, dci, 512 * tB : 512 * tB + 512],
                                    start=(dci == 0), stop=(dci == DC - 1),
                                )
                            c_sb = pcsb.tile([128, 512], BF16, tag="c_sb")
                            if tB % 2 == 0:
                                nc.scalar.activation(c_sb[:], psc[:], AF.Copy)
                            else:
                                nc.vector.tensor_copy(out=c_sb[:], in_=psc[:])
                            for cp, eng in ((0, nc.sync), (1, nc.scalar)):
                                dst = bass.AP(
                                    bufA[:].tensor,
                                    127 + 512 * tB + 2048 * cp,
                                    [[4223, 128], [1, 512]],
                                )
                                eng.dma_start(dst, c_sb[:])
                        wa = pwa.tile([128, 2560], BF16, tag="wa")
                        nc.sync.dma_start(
                            wa[:],
                            bass.AP(bufA[:].tensor, 128, [[4224, 128], [1, 2560]]),
                        )
                        pend.append((A, wa))
                        if len(pend) > 1:
                            _emit_mv(*pend.pop(0))
                    for a_w in pend:
                        _emit_mv(*a_w)
                    for cc in range(4):
                        nc.scalar.activation(
                            mvf[0:1, L * b + 512 * cc : L * b + 512 * cc + 512],
                            mv_reg[cc][0:1, :], AF.Copy,
                        )

                    # per-batch AllReduce, issued as soon as this mv is done
                    cci_b = dr.tile([1, L], F32, tag=f"cci{b}")
                    cco_b = dr.tile([1, L], F32, tag=f"cco{b}")
                    nc.gpsimd.dma_start(cci_b[:], mvf[0:1, L * b : L * b + L])
                    _selfcc = _os_env.environ.get("KERNEL_SELFCC", "0") == "1"
                    _nocc = _os_env.environ.get("KERNEL_NOCC", "0") == "1"
                    if _nocc:
                        nc.gpsimd.dma_start(cco_b[:], cci_b[:])
                    else:
                        nc.gpsimd.collective_compute(
                            "AllReduce", AluOpType.add,
                            replica_groups=(
                                [[c] for c in range(n_group)] if _selfcc
                                else [list(range(n_group))]
                            ),
                            ins=[cci_b[:].opt()], outs=[cco_b[:].opt()],
                        )
                    cco.append(cco_b)
                    if not _kp("KP1"):
                        _xwproj()

            # ---- phase 2: combine ARs, topk, masked softmax, band bufs
            with (
                tc.tile_pool(name="p12", bufs=1) as p12,
                tc.tile_pool(name="ppw", bufs=1, space="PSUM") as ppw,
            ):
                bsum = p12.tile([1, L], F32, tag="bsum")
                work = p12.tile([1, L], F32, tag="work")
                mask = p12.tile([1, L], F32, tag="mask")
                nbias = p12.tile([1, 1], F32, tag="nbias")
                nc.vector.memset(nbias[:], -1.0e4)
                pwm = ppw.tile([128, 512], F32, tag="pwm")

                def _warm(t):
                    # dummy matmul keyed on a just-written [1,*] tile: keeps
                    # the PE HAM window busy through the dependency-bound gap
                    nc.tensor.matmul(
                        pwm[:], lhsT=t[0:1, 0:128], rhs=t[0:1, 0:512],
                        start=True, stop=True,
                    )

                nc.gpsimd.dma_start(bsum[:], cco[0][:])
                nc.gpsimd.dma_start(work[:], cco[1][:])
                nc.vector.tensor_add(out=bsum[:], in0=bsum[:], in1=work[:])
                _warm(bsum)

                t_on = bsum
                for r, kk in enumerate((8, 8, TOPK - 16)):
                    mx8 = p12.tile([1, 8], F32, tag=f"mx8_{r}")
                    nc.vector.max(out=mx8[:], in_=t_on[:])
                    if kk < 8:
                        nc.vector.memset(mx8[:, kk:8], NEG)
                    nc.vector.match_replace(
                        out=work[:], in_to_replace=mx8[:], in_values=t_on[:],
                        imm_value=NEG,
                    )
                    t_on = work
                    _warm(work)
                nc.vector.tensor_sub(out=mask[:], in0=bsum[:], in1=work[:])
                nc.vector.tensor_scalar_min(mask[:], mask[:], 1.0)
                _warm(mask)

                for b in range(BPC):
                    # softmax over the 22 kept lags: gf = (mv+1e4)*mask,
                    # exp(gf-1e4) -> masked-out lanes underflow to exact 0
                    gf = bsum  # bsum is dead after the mask; reuse its slot
                    nc.vector.scalar_tensor_tensor(
                        out=gf[:], in0=mvf[0:1, L * b : L * b + L],
                        scalar=1.0e4, in1=mask[:],
                        op0=AluOpType.add, op1=AluOpType.mult,
                    )
                    nc.scalar.activation(gf[:], gf[:], AF.Exp, bias=nbias[0:1, 0:1])
                    _warm(gf)
                    zz = p12.tile([1, 1], F32, tag="sm_z")
                    nc.vector.reduce_sum(
                        out=zz[:], in_=gf[:], axis=mybir.AxisListType.X
                    )
                    nc.vector.reciprocal(out=zz[:], in_=zz[:])
                    gfb = p12.tile([1, L], BF16, tag=f"gfb{b}")
                    nc.vector.tensor_scalar_mul(gfb[:], gf[:], zz[:])
                    # periodic replication into DRAM; a row-step-2047 read
                    # yields the circulant band gbuf[p,m] = g[(127-p+m)%L].
                    # write and read are split across the sync+scalar rings
                    # to halve the serial DMA latency on the critical path.
                    hbuf = dr.tile([1, 129 * L], BF16, tag=f"hb{b}")
                    _gs = gfb[:]
                    _ga = [list(p) for p in _gs.ap]
                    hview = hbuf[:].rearrange("a (r n) -> a r n", r=129)
                    nc.sync.dma_start(
                        hview[:, 0:65, :],
                        bass.AP(_gs.tensor, _gs.offset,
                                [_ga[0], [0, 65], _ga[-1]]),
                    )
                    nc.scalar.dma_start(
                        hview[:, 65:129, :],
                        bass.AP(_gs.tensor, _gs.offset,
                                [_ga[0], [0, 64], _ga[-1]]),
                    )
                    hb.append(hbuf)
                    gbuf = pgbuf.tile([128, 3968], BF16, tag="gbuf",
                                      name="gbuf")
                    nc.sync.dma_start(
                        gbuf[0:64, :],
                        bass.AP(hbuf[:].tensor, 127, [[2047, 64], [1, 3968]]),
                    )
                    nc.scalar.dma_start(
                        gbuf[64:128, :],
                        bass.AP(hbuf[:].tensor, 127 + 2047 * 64,
                                [[2047, 64], [1, 3968]]),
                    )
                    gbufs.append(gbuf)

            pmv_ctx.__exit__(None, None, None)
            # ============ phase 3+4 per batch: circulant + decomp ==========
            with (
                tc.tile_pool(name="pacx", bufs=2) as pacx,
                tc.tile_pool(name="pxr", bufs=3) as pxr,
            ):
                for b in range(BPC):
                    gbuf = gbufs[b]
                    acx = pacx.tile([128, DC, L], BF16, tag="acx")
                    for dm in range(DC if _kp("KP3") else 0):
                        for nw in range(TW):
                            ps = pp.tile([128, 512], F32, tag="ps")
                            for Bc in range(TM):
                                gp = 512 * nw - 128 * Bc + 1920
                                nc.tensor.matmul(
                                    ps[:],
                                    lhsT=xw[b][:, Bc, 128 * dm : 128 * dm + 128],
                                    rhs=gbuf[:, gp : gp + 512],
                                    start=(Bc == 0), stop=(Bc == TM - 1),
                                )
                            xr = pxr.tile([128, 512], F32, tag="xr")
                            nc.sync.dma_start(
                                xr[:],
                                x_dm.ap()[b, 128 * dm : 128 * dm + 128,
                                          512 * nw : 512 * nw + 512],
                            )
                            nc.vector.scalar_tensor_tensor(
                                out=acx[:, dm, 512 * nw : 512 * nw + 512],
                                in0=ps[:], scalar=bop_c[:, dm : dm + 1], in1=xr[:],
                                op0=AluOpType.add, op1=AluOpType.add,
                            )
                    sb = pseab.tile([128, DC, L + 2], BF16, tag="seab")
                    if _kp("KP4"):
                        _decompose(nc, pscan, acx, sb)
                    seab.append(sb)

            pgbuf_ctx.__exit__(None, None, None)

        # ============ conv + decomp2 + layernorm, interleaved =============
        with (
            tc.tile_pool(name="pw1", bufs=4) as pw1,
            tc.tile_pool(name="pw2", bufs=2) as pw2,
            tc.tile_pool(name="ph1r", bufs=1) as ph1r,
            tc.tile_pool(name="pc5", bufs=2) as pc5,
            tc.tile_pool(name="pln", bufs=1) as pln,
            tc.tile_pool(name="pog", bufs=2) as pog,
            tc.tile_pool(name="ppm7", bufs=2, space="PSUM") as ppm7,
        ):
            # h1 ring: 2 self-contained window slots [left halo | 512 | right]
            h1s = [
                ph1r.tile([128, CFC, 514], BF16, tag=f"h1s{s}", name=f"h1s_{s}")
                for s in range(2)
            ]

            def conv1_win(b, nw):
                slot = h1s[nw % 2]
                sb = seab[b]
                for co in range(CFC if _kp("KP5") else 0):
                    w1t = pw1.tile([128, DC * 3, 128], BF16, tag="w1t")
                    nc.sync.dma_start(w1t[:], w1_d.ap()[co])
                    ps = pp.tile([128, 512], F32, tag="ps")
                    first = True
                    for dci in range(DC):
                        for tap in range(3):
                            nc.tensor.matmul(
                                ps[:],
                                lhsT=w1t[:, 3 * dci + tap, :],
                                rhs=sb[:, dci,
                                       512 * nw + tap : 512 * nw + tap + 512],
                                start=first, stop=(dci == DC - 1 and tap == 2),
                            )
                            first = False
                    nc.scalar.activation(
                        slot[:, co, 1:513], ps[:], AF.Lrelu, alpha=SLOPE
                    )
                # halo columns
                if nw == 0:
                    nc.vector.tensor_copy(out=slot[:, :, 0:1], in_=slot[:, :, 1:2])
                else:
                    nc.vector.tensor_copy(
                        out=slot[:, :, 0:1], in_=h1s[(nw - 1) % 2][:, :, 512:513]
                    )
                    nc.vector.tensor_copy(
                        out=h1s[(nw - 1) % 2][:, :, 513:514], in_=slot[:, :, 1:2]
                    )
                if nw == TW - 1:
                    nc.vector.tensor_copy(
                        out=slot[:, :, 513:514], in_=slot[:, :, 512:513]
                    )

            def conv2_win(b, nw, ysb, post_co=None):
                slot = h1s[nw % 2]
                for co in range(DC if _kp("KP6") else 0):
                    ps = pp.tile([128, 512], F32, tag="ps")
                    first = True
                    for hw in range(2):
                        w2t = pw2.tile([128, CFC * 3 // 2, 128], BF16, tag="w2t")
                        nc.scalar.dma_start(w2t[:], w2_d.ap()[co, :, hw])
                        for k in range(CFC * 3 // 2):
                            ci, tap = divmod(hw * CFC * 3 // 2 + k, 3)
                            nc.tensor.matmul(
                                ps[:],
                                lhsT=w2t[:, k, :],
                                rhs=slot[:, ci, tap : tap + 512],
                                start=first,
                                stop=(hw == 1 and k == CFC * 3 // 2 - 1),
                            )
                            first = False
                    h2r = pc5.tile([128, 512], F32, tag="h2r")
                    nc.scalar.activation(h2r[:], ps[:], AF.Lrelu, alpha=SLOPE)
                    nc.vector.tensor_add(
                        out=ysb[:, co, 512 * nw : 512 * nw + 512],
                        in0=h2r[:],
                        in1=seab[b][:, co, 1 + 512 * nw : 513 + 512 * nw],
                    )
                    if post_co is not None:
                        post_co(co)

            def conv_batch(b, ysb, post_co=None):
                conv1_win(b, 0)
                conv1_win(b, 1)
                conv2_win(b, 0, ysb)
                conv1_win(b, 2)
                conv2_win(b, 1, ysb)
                conv1_win(b, 3)
                conv2_win(b, 2, ysb)
                conv2_win(b, 3, ysb, post_co=post_co)

            def phase7(b, ysb, sea2):
                if _kp("KP7"):
                    _decompose(nc, pscan, ysb, sea2)
                stats = pln.tile([1, 2 * L], F32, tag="stats")
                for twi in range(TW if _kp("KP7") else 0):
                    st_s = ppm7.tile([1, 512], F32, tag="st_s")
                    st_q = ppm7.tile([1, 512], F32, tag="st_q")
                    for dci in range(DC):
                        sqt = pc5.tile([128, 512], BF16, tag="sqt")
                        nc.scalar.activation(
                            sqt[:],
                            sea2[:, dci, 1 + 512 * twi : 513 + 512 * twi],
                            AF.Square,
                        )
                        nc.tensor.matmul(
                            st_s[0:1, :], lhsT=ones_bf[:],
                            rhs=sea2[:, dci, 1 + 512 * twi : 513 + 512 * twi],
                            start=(dci == 0), stop=(dci == DC - 1),
                        )
                        nc.tensor.matmul(
                            st_q[0:1, :], lhsT=ones_bf[:], rhs=sqt[:],
                            start=(dci == 0), stop=(dci == DC - 1),
                        )
                    nc.scalar.activation(
                        stats[0:1, 512 * twi : 512 * twi + 512],
                        st_s[0:1, :], AF.Copy,
                    )
                    nc.scalar.activation(
                        stats[0:1, L + 512 * twi : L + 512 * twi + 512],
                        st_q[0:1, :], AF.Copy,
                    )
                if _kp("KP7"):
                    # fold stats [1,2L] -> [128,16]x2 via DRAM for fast rsqrt
                    st_d = dr.tile([1, 2 * L], F32, tag=f"st_d{b}")
                    nc.sync.dma_start(st_d[:], stats[:])
                    muf = pln.tile([128, 16], F32, tag="muf")
                    msf = pln.tile([128, 16], F32, tag="msf")
                    nc.sync.dma_start(
                        muf[:], bass.AP(st_d[:].tensor, 0, [[16, 128], [1, 16]])
                    )
                    nc.sync.dma_start(
                        msf[:], bass.AP(st_d[:].tensor, L, [[16, 128], [1, 16]])
                    )
                    varf = pln.tile([128, 16], F32, tag="varf")
                    nc.vector.tensor_mul(out=varf[:], in0=muf[:], in1=muf[:])
                    nc.vector.tensor_sub(out=varf[:], in0=msf[:], in1=varf[:])
                    nc.vector.tensor_scalar_add(varf[:], varf[:], EPS)
                    nc.vector.reciprocal(out=varf[:], in_=varf[:])
                    nc.scalar.activation(varf[:], varf[:], AF.Sqrt)
                    rs_d = dr.tile([1, L], F32, tag=f"rs_d{b}")
                    nc.sync.dma_start(
                        bass.AP(rs_d[:].tensor, 0, [[16, 128], [1, 16]]), varf[:]
                    )
                    mub = pln.tile([128, L], F32, tag="mub")
                    rsb = pln.tile([128, L], F32, tag="rsb")
                    nc.sync.dma_start(
                        mub[:], bass.AP(st_d[:].tensor, 0, [[0, 128], [1, L]])
                    )
                    nc.sync.dma_start(
                        rsb[:], bass.AP(rs_d[:].tensor, 0, [[0, 128], [1, L]])
                    )
                for dci in range(DC if _kp("KP7") else 0):
                    eng = nc.vector
                    og = pog.tile([128, L], BF16, tag="og")
                    eng.tensor_sub(
                        out=og[:], in0=sea2[:, dci, 1 : L + 1], in1=mub[:]
                    )
                    eng.tensor_mul(out=og[:], in0=og[:], in1=rsb[:])
                    nc.scalar.activation(
                        og[:], og[:], AF.Identity,
                        bias=lnb_c[:, dci : dci + 1], scale=lng_c[:, dci : dci + 1],
                    )
                    nc.scalar.dma_start(
                        out_dm.ap()[b, 128 * dci : 128 * dci + 128, :], og[:]
                    )

            ysb0 = pysb.tile([128, DC, L], BF16, tag="ysb")
            sea2_0 = psea2.tile([128, DC, L + 2], BF16, tag="sea2",
                                name="sea2_0")
            conv_batch(0, ysb0)
            phase7(0, ysb0, sea2_0)
            ysb1 = pysb.tile([128, DC, L], BF16, tag="ysb")
            sea2_1 = psea2.tile([128, DC, L + 2], BF16, tag="sea2",
                                name="sea2_1")
            conv_batch(1, ysb1)
            phase7(1, ysb1, sea2_1)


# ---------------------------------------------------------------------------
# host side
# ---------------------------------------------------------------------------
_CACHE = {}


def _get_nc(n_group: int):
    if n_group not in _CACHE:
        nc = bacc.Bacc("TRN2", target_bir_lowering=False, debug=False,
                       num_devices=n_group)
        build(nc, n_group)
        nc.compile()
        _CACHE[n_group] = nc
    return _CACHE[n_group]


def stage_inputs(inputs, ncores=NCORES):
    x = np.asarray(inputs["x"], np.float32)
    Wq = np.asarray(inputs["Wq"], np.float32)
    Wk = np.asarray(inputs["Wk"], np.float32)
    Wv = np.asarray(inputs["Wv"], np.float32)
    Wo = np.asarray(inputs["Wo"], np.float32)
    bv = np.asarray(inputs["bv"], np.float32)
    bo = np.asarray(inputs["bo"], np.float32)
    w1 = np.asarray(inputs["conv1_w"], np.float32)
    w2 = np.asarray(inputs["conv2_w"], np.float32)
    lng = np.asarray(inputs["ln_g"], np.float32)
    lnb = np.asarray(inputs["ln_b"], np.float32)

    bop = bo + bv @ Wo
    col = lambda v: np.ascontiguousarray(v.reshape(DC, 128).T)
    dmaj = lambda M: np.ascontiguousarray(
        M.reshape(DC, 128, D).transpose(1, 0, 2)
    ).astype(BF16_NP)
    # corr = x^T (Wq Wk^T) x  ->  xg = (Wq Wk^T) x, staged pre-transposed
    gt_h = dmaj(Wk @ Wq.T)
    # rolls commute with channel mixing: fold Wv@Wo
    wvo_h = dmaj(Wv @ Wo)
    w1s = np.ascontiguousarray(
        w1.reshape(3, DC, 128, CFC, 128).transpose(3, 2, 1, 0, 4)
    ).reshape(CFC, 128, DC * 3, 128).astype(BF16_NP)
    # w2 staged as [co, p, 48, 128] with the 48 (ci,tap) pairs in order,
    # then split into two halves of 24 for streaming
    w2s = np.ascontiguousarray(
        w2.reshape(3, CFC, 128, DC, 128).transpose(3, 2, 1, 0, 4)
    ).reshape(DC, 128, 2, CFC * 3 // 2, 128).astype(BF16_NP)

    shared = {
        "gt_h": gt_h, "wvo_h": wvo_h, "bop_t": col(bop),
        "w1s": w1s, "w2s": w2s, "lng_t": col(lng), "lnb_t": col(lnb),
    }
    bpc = B // ncores
    in_maps = []
    for c in range(ncores):
        m = dict(shared)
        xc = np.ascontiguousarray(x[bpc * c : bpc * (c + 1)].transpose(0, 2, 1))
        m["x_dm"] = xc
        m["xh_bf"] = np.ascontiguousarray(
            xc.reshape(bpc, DC, 128, L).transpose(0, 2, 1, 3)
        ).astype(BF16_NP)
        in_maps.append(m)
    return in_maps


def kernel(**inputs):
    nc = _get_nc(NCORES)
    in_maps = stage_inputs(inputs)
    res = bass_utils.run_bass_kernel_spmd(nc, in_maps, core_ids=list(range(NCORES)))
    out = np.empty((B, L, D), np.float32)
    for c in range(NCORES):
        o = np.asarray(res.results[c]["out_dm"])  # [BPC, D, L] bf16
        for i in range(BPC):
            out[BPC * c + i] = o[i].T.astype(np.float32)
    return out


# revision 19
# speedup vs baseline: 1.0249x; 1.0001x over previous
"""Trainium2 Bass kernel for nn_Encoder_78889959293176 (Autoformer-style encoder).

Data-parallel over batch (16 batches -> 8 cores x 2). v2 layout:
  - host-folded weights: G = Wq@Wk^T (corr = x^T G x), Wvo = Wv@Wo
    (rolls commute with channel mixing), removing the q/k/v projection
    passes entirely.
  - correlation statistic via x_A^T (Gx) tiles + 2-copy diagonal shear
    through DRAM + ones-matmul reduction (unchanged mechanism).
  - AllReduce split in two (one per local batch) so the first hides
    under the second batch's correlation; on-device top-22 + masked
    softmax -> circulant band buffer (broadcast DMA trick).
  - conv1/conv2 fused per 512-col window with a 2-slot h1 ring
    (SBUF), conv weights streamed from HBM in host-pretransposed
    per-partition-contiguous layout.
  - both batches interleaved in emission order so DVE phases
    (decomposition, layernorm) hide under the other batch's matmuls.
"""

import numpy as np

import concourse.bass as bass
import concourse.bacc as bacc
import concourse.mybir as mybir
import concourse.tile as tile
from concourse import bass_utils
from concourse.alu_op_type import AluOpType

try:
    import ml_dtypes

    BF16_NP = ml_dtypes.bfloat16
except Exception:  # pragma: no cover
    BF16_NP = np.float32

F32 = mybir.dt.float32
BF16 = mybir.dt.bfloat16
AF = mybir.ActivationFunctionType

B, L, D = 16, 2048, 512
CF = 2048
TOPK = 22
KER = 25
EPS = 1e-5
SLOPE = 0.01
NCORES = 8
BPC = B // NCORES
DC = D // 128  # 4
CFC = CF // 128  # 16
TW = L // 512  # 4
TM = L // 128  # 16
NEG = -1.0e30

import os as _os_env


def _kp(name):
    return _os_env.environ.get(name, "1") == "1"


def build(nc: bass.Bass, n_group: int, lite: bool = False):
    x_dm = nc.dram_tensor("x_dm", [BPC, D, L], F32, kind="ExternalInput")
    xh_dm = nc.dram_tensor("xh_bf", [BPC, 128, DC, L], BF16, kind="ExternalInput")
    gt_d = nc.dram_tensor("gt_h", [128, DC, D], BF16, kind="ExternalInput")
    wvo_d = nc.dram_tensor("wvo_h", [128, DC, D], BF16, kind="ExternalInput")
    bop_d = nc.dram_tensor("bop_t", [128, DC], F32, kind="ExternalInput")
    w1_d = nc.dram_tensor("w1s", [CFC, 128, DC * 3, 128], BF16, kind="ExternalInput")
    w2_d = nc.dram_tensor("w2s", [DC, 128, 2, CFC * 3 // 2, 128], BF16,
                          kind="ExternalInput")
    lng_d = nc.dram_tensor("lng_t", [128, DC], F32, kind="ExternalInput")
    lnb_d = nc.dram_tensor("lnb_t", [128, DC], F32, kind="ExternalInput")
    out_dm = nc.dram_tensor("out_dm", [BPC, D, L], BF16, kind="ExternalOutput")

    with tile.TileContext(nc) as tc:
        _body(nc, tc, n_group, x_dm, xh_dm, gt_d, wvo_d, bop_d, w1_d, w2_d,
              lng_d, lnb_d, out_dm)
    return nc


def _decompose(nc, scan_pool, src, dst):
    for dci in range(DC):
        _decompose_dci(nc, scan_pool, src, dst, dci)


def _decompose_dci(nc, scan_pool, src, dst, dci):
    """dst[:, dci, 1:L+1] = src[:, dci] - movavg_KER; replicated edge cols."""
    half = (KER - 1) // 2
    if True:
        eng = nc.vector
        pad = scan_pool.tile([128, L + KER], F32, tag="scan_pad",
                             name="scan_pad")  # noqa
        cs = scan_pool.tile([128, L + KER], F32, tag="scan_cs",
                            name="scan_cs")
        eng.memset(pad[:, 0:1], 0.0)
        eng.tensor_copy(
            out=pad[:, 1 : 1 + half],
            in_=src[:, dci, 0:1].to_broadcast([128, half]),
        )
        nc.scalar.activation(pad[:, 1 + half : 1 + half + L], src[:, dci, :], AF.Copy)
        eng.tensor_copy(
            out=pad[:, 1 + half + L :],
            in_=src[:, dci, L - 1 : L].to_broadcast([128, half]),
        )
        eng.tensor_tensor_scan(
            out=cs[:], data0=pad[:], data1=pad[:], initial=0.0,
            op0=AluOpType.add, op1=AluOpType.bypass,
        )
        # d1 reuses pad (dead after the scan)
        eng.tensor_sub(out=pad[:, 0:L], in0=cs[:, KER:], in1=cs[:, 0:L])
        eng.scalar_tensor_tensor(
            out=dst[:, dci, 1 : L + 1], in0=pad[:, 0:L], scalar=-1.0 / KER,
            in1=src[:, dci, :], op0=AluOpType.mult, op1=AluOpType.add,
        )
        eng.tensor_copy(out=dst[:, dci, 0:1], in_=dst[:, dci, 1:2])
        eng.tensor_copy(
            out=dst[:, dci, L + 1 : L + 2], in_=dst[:, dci, L : L + 1]
        )


def _body(nc, tc, n_group, x_dm, xh_dm, gt_d, wvo_d, bop_d, w1_d, w2_d,
          lng_d, lnb_d, out_dm):
    with (
        tc.tile_pool(name="p0", bufs=1) as p0,
        tc.tile_pool(name="pp", bufs=4, space="PSUM") as pp,
        tc.tile_pool(name="dr", bufs=1, space="DRAM") as dr,
        tc.tile_pool(name="dr3", bufs=4, space="DRAM") as dr3,
        tc.tile_pool(name="pseab", bufs=2) as pseab,
        tc.tile_pool(name="pysb", bufs=1) as pysb,
        tc.tile_pool(name="psea2", bufs=1) as psea2,
        tc.tile_pool(name="pscan", bufs=1) as pscan,
    ):
        # ---------------- persistent constants ----------------
        ones_bf = p0.tile([128, 1], BF16, tag="ones_bf")
        nc.vector.memset(ones_bf[:], 1.0 / D)
        bop_c = p0.tile([128, DC], F32, tag="bop_c")
        lng_c = p0.tile([128, DC], F32, tag="lng_c")
        lnb_c = p0.tile([128, DC], F32, tag="lnb_c")
        nc.sync.dma_start(bop_c[:], bop_d[:, :])
        nc.sync.dma_start(lng_c[:], lng_d[:, :])
        nc.sync.dma_start(lnb_c[:], lnb_d[:, :])
        hb = []
        seab = []

        with tc.tile_pool(name="pxw", bufs=2) as pxw:
            xw = []
            gt_s = pxw.tile([128, DC, D], BF16, tag="gt_s")
            wvo_s = pxw.tile([128, DC, D], BF16, tag="wvo_s")
            nc.sync.dma_start(gt_s[:], gt_d[:, :, :])
            nc.sync.dma_start(wvo_s[:], wvo_d[:, :, :])
            pgbuf_ctx = tc.tile_pool(name="pgbuf", bufs=2)
            pgbuf = pgbuf_ctx.__enter__()
            gbufs = []
            pmv_ctx = tc.tile_pool(name="pmv", bufs=1)
            pmv = pmv_ctx.__enter__()
            mvf = pmv.tile([1, BPC * L], F32, tag="mvf")
            cco = []
            # ============ phase 1: mean_value + per-batch AllReduce ========
            with (
                tc.tile_pool(name="pxb", bufs=1) as pxb,
                tc.tile_pool(name="pxg", bufs=1) as pxg,
                tc.tile_pool(name="pwa", bufs=2) as pwa,
                tc.tile_pool(name="pcsb", bufs=2) as pcsb,
                tc.tile_pool(name="ppm1", bufs=1, space="PSUM") as ppm1,
            ):
                xbs = []
                for b in range(BPC):
                    xb = pxb.tile([128, DC, L], BF16, tag="xb")
                    if b == 0:
                        for w in range(TW):
                            nc.sync.dma_start(
                                xb[:, :, 512 * w : 512 * w + 512],
                                xh_dm.ap()[b, :, :, 512 * w : 512 * w + 512],
                            )
                    else:
                        nc.sync.dma_start(xb[:], xh_dm.ap()[b])
                    xbs.append(xb)

                    # xg = (Wq Wk^T) x  (d-major)
                    xg = pxg.tile([128, DC, L], BF16, tag="xg")
                    for dco in range(DC):
                        for twi in range(TW):
                            ps = pp.tile([128, 512], F32, tag="ps")
                            for dci in range(DC):
                                nc.tensor.matmul(
                                    ps[:],
                                    lhsT=gt_s[:, dci, 128 * dco : 128 * dco + 128],
                                    rhs=xb[:, dci, 512 * twi : 512 * twi + 512],
                                    start=(dci == 0), stop=(dci == DC - 1),
                                )
                            if (dco + twi) % 2 == 0:
                                nc.scalar.activation(
                                    xg[:, dco, 512 * twi : 512 * twi + 512],
                                    ps[:], AF.Copy,
                                )
                            else:
                                nc.vector.tensor_copy(
                                    out=xg[:, dco, 512 * twi : 512 * twi + 512],
                                    in_=ps[:],
                                )

                    # corr tiles + diagonal shear + ones-matmul lag reduction
                    mv_reg = [
                        ppm1.tile([1, 512], F32, tag=f"mv{cc}", name=f"mv_{cc}")
                        for cc in range(4)
                    ]

                    def _emit_mv(A, wa, mv_reg=mv_reg):
                        for cc in range(4):
                            w0 = (512 * cc + 128 * A) % L
                            nc.tensor.matmul(
                                mv_reg[cc][0:1, :],
                                lhsT=ones_bf[:],
                                rhs=wa[:, w0 : w0 + 512],
                                start=(A == 0), stop=(A == TM - 1),
                            )

                    def _xwproj(xb=xb):
                        xw_b = pxw.tile([128, TM, D], BF16, tag="xw",
                                        name="xw_b")
                        for tm in range(TM if _kp("KP3") else 0):
                            ps = pp.tile([128, 512], F32, tag="ps")
                            for dci in range(DC):
                                nc.tensor.matmul(
                                    ps[:],
                                    lhsT=xb[:, dci, 128 * tm : 128 * tm + 128],
                                    rhs=wvo_s[:, dci, :],
                                    start=(dci == 0), stop=(dci == DC - 1),
                                )
                            if tm % 2 == 0:
                                nc.scalar.activation(xw_b[:, tm, :], ps[:], AF.Copy)
                            else:
                                nc.vector.tensor_copy(out=xw_b[:, tm, :], in_=ps[:])
                        xw.append(xw_b)

                    pend = []
                    for A in range(TM if _kp("KP1") else 0):
                        if A == 8:
                            _xwproj()  # frees the xb slot before the next
                            # batch's load; fills PE during the shear tail
                        bufA = dr3.tile([128, 4224], BF16, tag="bufA")
                        for tB in range(TW):
                            psc = pp.tile([128, 512], F32, tag="ps")
                            for dci in range(DC):
                                nc.tensor.matmul(
                                    psc[:],
                                    lhsT=xb[:, dci, 128 * A : 128 * A + 128],
                                    rhs=xg[:, dci, 512 * tB : 512 * tB + 512],
                                    start=(dci == 0), stop=(dci == DC - 1),
                                )
                            c_sb = pcsb.tile([128, 512], BF16, tag="c_sb")
                            if tB % 2 == 0:
                                nc.scalar.activation(c_sb[:], psc[:], AF.Copy)
                            else:
                                nc.vector.tensor_copy(out=c_sb[:], in_=psc[:])
                            for cp, eng in ((0, nc.sync), (1, nc.scalar)):
                                dst = bass.AP(
                                    bufA[:].tensor,
                                    127 + 512 * tB + 2048 * cp,
                                    [[4223, 128], [1, 512]],
                                )
                                eng.dma_start(dst, c_sb[:])
                        wa = pwa.tile([128, 2560], BF16, tag="wa")
                        nc.sync.dma_start(
                            wa[:],
                            bass.AP(bufA[:].tensor, 128, [[4224, 128], [1, 2560]]),
                        )
                        pend.append((A, wa))
                        if len(pend) > 1:
                            _emit_mv(*pend.pop(0))
                    for a_w in pend:
                        _emit_mv(*a_w)
                    for cc in range(4):
                        nc.scalar.activation(
                            mvf[0:1, L * b + 512 * cc : L * b + 512 * cc + 512],
                            mv_reg[cc][0:1, :], AF.Copy,
                        )

                    # per-batch AllReduce, issued as soon as this mv is done
                    cci_b = dr.tile([1, L], F32, tag=f"cci{b}")
                    cco_b = dr.tile([1, L], F32, tag=f"cco{b}")
                    nc.gpsimd.dma_start(cci_b[:], mvf[0:1, L * b : L * b + L])
                    _selfcc = _os_env.environ.get("KERNEL_SELFCC", "0") == "1"
                    _nocc = _os_env.environ.get("KERNEL_NOCC", "0") == "1"
                    if _nocc:
                        nc.gpsimd.dma_start(cco_b[:], cci_b[:])
                    else:
                        nc.gpsimd.collective_compute(
                            "AllReduce", AluOpType.add,
                            replica_groups=(
                                [[c] for c in range(n_group)] if _selfcc
                                else [list(range(n_group))]
                            ),
                            ins=[cci_b[:].opt()], outs=[cco_b[:].opt()],
                        )
                    cco.append(cco_b)
                    if not _kp("KP1"):
                        _xwproj()

            # ---- phase 2: combine ARs, topk, masked softmax, band bufs
            with (
                tc.tile_pool(name="p12", bufs=1) as p12,
                tc.tile_pool(name="ppw", bufs=1, space="PSUM") as ppw,
            ):
                bsum = p12.tile([1, L], F32, tag="bsum")
                work = p12.tile([1, L], F32, tag="work")
                mask = p12.tile([1, L], F32, tag="mask")
                nbias = p12.tile([1, 1], F32, tag="nbias")
                nc.vector.memset(nbias[:], -1.0e4)
                pwm = ppw.tile([128, 512], F32, tag="pwm")

                def _warm(t):
                    # dummy matmul keyed on a just-written [1,*] tile: keeps
                    # the PE HAM window busy through the dependency-bound gap
                    nc.tensor.matmul(
                        pwm[:], lhsT=t[0:1, 0:128], rhs=t[0:1, 0:512],
                        start=True, stop=True,
                    )

                nc.gpsimd.dma_start(bsum[:], cco[0][:])
                nc.gpsimd.dma_start(work[:], cco[1][:])
                nc.vector.tensor_add(out=bsum[:], in0=bsum[:], in1=work[:])
                _warm(bsum)

                t_on = bsum
                for r, kk in enumerate((8, 8, TOPK - 16)):
                    mx8 = p12.tile([1, 8], F32, tag=f"mx8_{r}")
                    nc.vector.max(out=mx8[:], in_=t_on[:])
                    if kk < 8:
                        nc.vector.memset(mx8[:, kk:8], NEG)
                    nc.vector.match_replace(
                        out=work[:], in_to_replace=mx8[:], in_values=t_on[:],
                        imm_value=NEG,
                    )
                    t_on = work
                    _warm(work)
                nc.vector.tensor_sub(out=mask[:], in0=bsum[:], in1=work[:])
                nc.vector.tensor_scalar_min(mask[:], mask[:], 1.0)
                _warm(mask)

                for b in range(BPC):
                    # softmax over the 22 kept lags: gf = (mv+1e4)*mask,
                    # exp(gf-1e4) -> masked-out lanes underflow to exact 0
                    gf = bsum  # bsum is dead after the mask; reuse its slot
                    nc.vector.scalar_tensor_tensor(
                        out=gf[:], in0=mvf[0:1, L * b : L * b + L],
                        scalar=1.0e4, in1=mask[:],
                        op0=AluOpType.add, op1=AluOpType.mult,
                    )
                    nc.scalar.activation(gf[:], gf[:], AF.Exp, bias=nbias[0:1, 0:1])
                    _warm(gf)
                    zz = p12.tile([1, 1], F32, tag="sm_z")
                    nc.vector.reduce_sum(
                        out=zz[:], in_=gf[:], axis=mybir.AxisListType.X
                    )
                    nc.vector.reciprocal(out=zz[:], in_=zz[:])
                    gfb = p12.tile([1, L], BF16, tag=f"gfb{b}")
                    nc.vector.tensor_scalar_mul(gfb[:], gf[:], zz[:])
                    # periodic replication into DRAM; a row-step-2047 read
                    # yields the circulant band gbuf[p,m] = g[(127-p+m)%L].
                    # b0 on the sync ring, b1 on scalar -> the two chains
                    # don't head-of-line block each other.
                    deng = nc.sync if b == 0 else nc.scalar
                    hbuf = dr.tile([1, 129 * L], BF16, tag=f"hb{b}")
                    _gs = gfb[:]
                    _ga = [list(p) for p in _gs.ap]
                    grep_ap = bass.AP(
                        _gs.tensor, _gs.offset, [_ga[0], [0, 129], _ga[-1]]
                    )
                    deng.dma_start(
                        hbuf[:].rearrange("a (r n) -> a r n", r=129), grep_ap
                    )
                    hb.append(hbuf)
                    gbuf = pgbuf.tile([128, 3968], BF16, tag="gbuf",
                                      name="gbuf")
                    deng.dma_start(
                        gbuf[:],
                        bass.AP(hbuf[:].tensor, 127, [[2047, 128], [1, 3968]]),
                    )
                    gbufs.append(gbuf)

            pmv_ctx.__exit__(None, None, None)
            # ============ phase 3+4 per batch: circulant + decomp ==========
            with (
                tc.tile_pool(name="pacx", bufs=2) as pacx,
                tc.tile_pool(name="pxr", bufs=3) as pxr,
            ):
                for b in range(BPC):
                    gbuf = gbufs[b]
                    acx = pacx.tile([128, DC, L], BF16, tag="acx")
                    for dm in range(DC if _kp("KP3") else 0):
                        for nw in range(TW):
                            ps = pp.tile([128, 512], F32, tag="ps")
                            for Bc in range(TM):
                                gp = 512 * nw - 128 * Bc + 1920
                                nc.tensor.matmul(
                                    ps[:],
                                    lhsT=xw[b][:, Bc, 128 * dm : 128 * dm + 128],
                                    rhs=gbuf[:, gp : gp + 512],
                                    start=(Bc == 0), stop=(Bc == TM - 1),
                                )
                            xr = pxr.tile([128, 512], F32, tag="xr")
                            nc.sync.dma_start(
                                xr[:],
                                x_dm.ap()[b, 128 * dm : 128 * dm + 128,
                                          512 * nw : 512 * nw + 512],
                            )
                            nc.vector.scalar_tensor_tensor(
                                out=acx[:, dm, 512 * nw : 512 * nw + 512],
                                in0=ps[:], scalar=bop_c[:, dm : dm + 1], in1=xr[:],
                                op0=AluOpType.add, op1=AluOpType.add,
                            )
                    sb = pseab.tile([128, DC, L + 2], BF16, tag="seab")
                    if _kp("KP4"):
                        _decompose(nc, pscan, acx, sb)
                    seab.append(sb)

            pgbuf_ctx.__exit__(None, None, None)

        # ============ conv + decomp2 + layernorm, interleaved =============
        with (
            tc.tile_pool(name="pw1", bufs=4) as pw1,
            tc.tile_pool(name="pw2", bufs=2) as pw2,
            tc.tile_pool(name="ph1r", bufs=1) as ph1r,
            tc.tile_pool(name="pc5", bufs=2) as pc5,
            tc.tile_pool(name="pln", bufs=1) as pln,
            tc.tile_pool(name="pog", bufs=2) as pog,
            tc.tile_pool(name="ppm7", bufs=2, space="PSUM") as ppm7,
        ):
            # h1 ring: 2 self-contained window slots [left halo | 512 | right]
            h1s = [
                ph1r.tile([128, CFC, 514], BF16, tag=f"h1s{s}", name=f"h1s_{s}")
                for s in range(2)
            ]

            def conv1_win(b, nw):
                slot = h1s[nw % 2]
                sb = seab[b]
                for co in range(CFC if _kp("KP5") else 0):
                    w1t = pw1.tile([128, DC * 3, 128], BF16, tag="w1t")
                    nc.sync.dma_start(w1t[:], w1_d.ap()[co])
                    ps = pp.tile([128, 512], F32, tag="ps")
                    first = True
                    for dci in range(DC):
                        for tap in range(3):
                            nc.tensor.matmul(
                                ps[:],
                                lhsT=w1t[:, 3 * dci + tap, :],
                                rhs=sb[:, dci,
                                       512 * nw + tap : 512 * nw + tap + 512],
                                start=first, stop=(dci == DC - 1 and tap == 2),
                            )
                            first = False
                    nc.scalar.activation(
                        slot[:, co, 1:513], ps[:], AF.Lrelu, alpha=SLOPE
                    )
                # halo columns
                if nw == 0:
                    nc.vector.tensor_copy(out=slot[:, :, 0:1], in_=slot[:, :, 1:2])
                else:
                    nc.vector.tensor_copy(
                        out=slot[:, :, 0:1], in_=h1s[(nw - 1) % 2][:, :, 512:513]
                    )
                    nc.vector.tensor_copy(
                        out=h1s[(nw - 1) % 2][:, :, 513:514], in_=slot[:, :, 1:2]
                    )
                if nw == TW - 1:
                    nc.vector.tensor_copy(
                        out=slot[:, :, 513:514], in_=slot[:, :, 512:513]
                    )

            def conv2_win(b, nw, ysb, post_co=None):
                slot = h1s[nw % 2]
                for co in range(DC if _kp("KP6") else 0):
                    ps = pp.tile([128, 512], F32, tag="ps")
                    first = True
                    for hw in range(2):
                        w2t = pw2.tile([128, CFC * 3 // 2, 128], BF16, tag="w2t")
                        nc.scalar.dma_start(w2t[:], w2_d.ap()[co, :, hw])
                        for k in range(CFC * 3 // 2):
                            ci, tap = divmod(hw * CFC * 3 // 2 + k, 3)
                            nc.tensor.matmul(
                                ps[:],
                                lhsT=w2t[:, k, :],
                                rhs=slot[:, ci, tap : tap + 512],
                                start=first,
                                stop=(hw == 1 and k == CFC * 3 // 2 - 1),
                            )
                            first = False
                    h2r = pc5.tile([128, 512], F32, tag="h2r")
                    nc.scalar.activation(h2r[:], ps[:], AF.Lrelu, alpha=SLOPE)
                    nc.vector.tensor_add(
                        out=ysb[:, co, 512 * nw : 512 * nw + 512],
                        in0=h2r[:],
                        in1=seab[b][:, co, 1 + 512 * nw : 513 + 512 * nw],
                    )
                    if post_co is not None:
                        post_co(co)

            def conv_batch(b, ysb, post_co=None):
                conv1_win(b, 0)
                conv1_win(b, 1)
                conv2_win(b, 0, ysb)
                conv1_win(b, 2)
                conv2_win(b, 1, ysb)
                conv1_win(b, 3)
                conv2_win(b, 2, ysb)
                conv2_win(b, 3, ysb, post_co=post_co)

            def phase7(b, ysb, sea2):
                if _kp("KP7"):
                    _decompose(nc, pscan, ysb, sea2)
                stats = pln.tile([1, 2 * L], F32, tag="stats")
                for twi in range(TW if _kp("KP7") else 0):
                    st_s = ppm7.tile([1, 512], F32, tag="st_s")
                    st_q = ppm7.tile([1, 512], F32, tag="st_q")
                    for dci in range(DC):
                        sqt = pc5.tile([128, 512], BF16, tag="sqt")
                        nc.scalar.activation(
                            sqt[:],
                            sea2[:, dci, 1 + 512 * twi : 513 + 512 * twi],
                            AF.Square,
                        )
                        nc.tensor.matmul(
                            st_s[0:1, :], lhsT=ones_bf[:],
                            rhs=sea2[:, dci, 1 + 512 * twi : 513 + 512 * twi],
                            start=(dci == 0), stop=(dci == DC - 1),
                        )
                        nc.tensor.matmul(
                            st_q[0:1, :], lhsT=ones_bf[:], rhs=sqt[:],
                            start=(dci == 0), stop=(dci == DC - 1),
                        )
                    nc.scalar.activation(
                        stats[0:1, 512 * twi : 512 * twi + 512],
                        st_s[0:1, :], AF.Copy,
                    )
                    nc.scalar.activation(
                        stats[0:1, L + 512 * twi : L + 512 * twi + 512],
                        st_q[0:1, :], AF.Copy,
                    )
                if _kp("KP7"):
                    # fold stats [1,2L] -> [128,16]x2 via DRAM for fast rsqrt
                    st_d = dr.tile([1, 2 * L], F32, tag=f"st_d{b}")
                    nc.sync.dma_start(st_d[:], stats[:])
                    muf = pln.tile([128, 16], F32, tag="muf")
                    msf = pln.tile([128, 16], F32, tag="msf")
                    nc.sync.dma_start(
                        muf[:], bass.AP(st_d[:].tensor, 0, [[16, 128], [1, 16]])
                    )
                    nc.sync.dma_start(
                        msf[:], bass.AP(st_d[:].tensor, L, [[16, 128], [1, 16]])
                    )
                    varf = pln.tile([128, 16], F32, tag="varf")
                    nc.vector.tensor_mul(out=varf[:], in0=muf[:], in1=muf[:])
                    nc.vector.tensor_sub(out=varf[:], in0=msf[:], in1=varf[:])
                    nc.vector.tensor_scalar_add(varf[:], varf[:], EPS)
                    nc.vector.reciprocal(out=varf[:], in_=varf[:])
                    nc.scalar.activation(varf[:], varf[:], AF.Sqrt)
                    rs_d = dr.tile([1, L], F32, tag=f"rs_d{b}")
                    nc.sync.dma_start(
                        bass.AP(rs_d[:].tensor, 0, [[16, 128], [1, 16]]), varf[:]
                    )
                    mub = pln.tile([128, L], F32, tag="mub")
                    rsb = pln.tile([128, L], F32, tag="rsb")
                    nc.sync.dma_start(
                        mub[:], bass.AP(st_d[:].tensor, 0, [[0, 128], [1, L]])
                    )
                    nc.sync.dma_start(
                        rsb[:], bass.AP(rs_d[:].tensor, 0, [[0, 128], [1, L]])
                    )
                for dci in range(DC if _kp("KP7") else 0):
                    eng = nc.vector
                    og = pog.tile([128, L], BF16, tag="og")
                    eng.tensor_sub(
                        out=og[:], in0=sea2[:, dci, 1 : L + 1], in1=mub[:]
                    )
                    eng.tensor_mul(out=og[:], in0=og[:], in1=rsb[:])
                    nc.scalar.activation(
                        og[:], og[:], AF.Identity,
                        bias=lnb_c[:, dci : dci + 1], scale=lng_c[:, dci : dci + 1],
                    )
                    nc.scalar.dma_start(
                        out_dm.ap()[b, 128 * dci : 128 * dci + 128, :], og[:]
                    )

            ysb0 = pysb.tile([128, DC, L], BF16, tag="ysb")
            sea2_0 = psea2.tile([128, DC, L + 2], BF16, tag="sea2",
                                name="sea2_0")
            conv_batch(0, ysb0)
            phase7(0, ysb0, sea2_0)
            ysb1 = pysb.tile([128, DC, L], BF16, tag="ysb")
            sea2_1 = psea2.tile([128, DC, L + 2], BF16, tag="sea2",
                                name="sea2_1")
            conv_batch(1, ysb1)
            phase7(1, ysb1, sea2_1)


# ---------------------------------------------------------------------------
# host side
# ---------------------------------------------------------------------------
_CACHE = {}


def _get_nc(n_group: int):
    if n_group not in _CACHE:
        nc = bacc.Bacc("TRN2", target_bir_lowering=False, debug=False,
                       num_devices=n_group)
        build(nc, n_group)
        nc.compile()
        _CACHE[n_group] = nc
    return _CACHE[n_group]


def stage_inputs(inputs, ncores=NCORES):
    x = np.asarray(inputs["x"], np.float32)
    Wq = np.asarray(inputs["Wq"], np.float32)
    Wk = np.asarray(inputs["Wk"], np.float32)
    Wv = np.asarray(inputs["Wv"], np.float32)
    Wo = np.asarray(inputs["Wo"], np.float32)
    bv = np.asarray(inputs["bv"], np.float32)
    bo = np.asarray(inputs["bo"], np.float32)
    w1 = np.asarray(inputs["conv1_w"], np.float32)
    w2 = np.asarray(inputs["conv2_w"], np.float32)
    lng = np.asarray(inputs["ln_g"], np.float32)
    lnb = np.asarray(inputs["ln_b"], np.float32)

    bop = bo + bv @ Wo
    col = lambda v: np.ascontiguousarray(v.reshape(DC, 128).T)
    dmaj = lambda M: np.ascontiguousarray(
        M.reshape(DC, 128, D).transpose(1, 0, 2)
    ).astype(BF16_NP)
    # corr = x^T (Wq Wk^T) x  ->  xg = (Wq Wk^T) x, staged pre-transposed
    gt_h = dmaj(Wk @ Wq.T)
    # rolls commute with channel mixing: fold Wv@Wo
    wvo_h = dmaj(Wv @ Wo)
    w1s = np.ascontiguousarray(
        w1.reshape(3, DC, 128, CFC, 128).transpose(3, 2, 1, 0, 4)
    ).reshape(CFC, 128, DC * 3, 128).astype(BF16_NP)
    # w2 staged as [co, p, 48, 128] with the 48 (ci,tap) pairs in order,
    # then split into two halves of 24 for streaming
    w2s = np.ascontiguousarray(
        w2.reshape(3, CFC, 128, DC, 128).transpose(3, 2, 1, 0, 4)
    ).reshape(DC, 128, 2, CFC * 3 // 2, 128).astype(BF16_NP)

    shared = {
        "gt_h": gt_h, "wvo_h": wvo_h, "bop_t": col(bop),
        "w1s": w1s, "w2s": w2s, "lng_t": col(lng), "lnb_t": col(lnb),
    }
    bpc = B // ncores
    in_maps = []
    for c in range(ncores):
        m = dict(shared)
        xc = np.ascontiguousarray(x[bpc * c : bpc * (c + 1)].transpose(0, 2, 1))
        m["x_dm"] = xc
        m["xh_bf"] = np.ascontiguousarray(
            xc.reshape(bpc, DC, 128, L).transpose(0, 2, 1, 3)
        ).astype(BF16_NP)
        in_maps.append(m)
    return in_maps


def kernel(**inputs):
    nc = _get_nc(NCORES)
    in_maps = stage_inputs(inputs)
    res = bass_utils.run_bass_kernel_spmd(nc, in_maps, core_ids=list(range(NCORES)))
    out = np.empty((B, L, D), np.float32)
    for c in range(NCORES):
        o = np.asarray(res.results[c]["out_dm"])  # [BPC, D, L] bf16
        for i in range(BPC):
            out[BPC * c + i] = o[i].T.astype(np.float32)
    return out


# revision 20
# speedup vs baseline: 1.0317x; 1.0067x over previous
"""Trainium2 Bass kernel for nn_Encoder_78889959293176 (Autoformer-style encoder).

Data-parallel over batch (16 batches -> 8 cores x 2). v2 layout:
  - host-folded weights: G = Wq@Wk^T (corr = x^T G x), Wvo = Wv@Wo
    (rolls commute with channel mixing), removing the q/k/v projection
    passes entirely.
  - correlation statistic via x_A^T (Gx) tiles + 2-copy diagonal shear
    through DRAM + ones-matmul reduction (unchanged mechanism).
  - AllReduce split in two (one per local batch) so the first hides
    under the second batch's correlation; on-device top-22 + masked
    softmax -> circulant band buffer (broadcast DMA trick).
  - conv1/conv2 fused per 512-col window with a 2-slot h1 ring
    (SBUF), conv weights streamed from HBM in host-pretransposed
    per-partition-contiguous layout.
  - both batches interleaved in emission order so DVE phases
    (decomposition, layernorm) hide under the other batch's matmuls.
"""

import numpy as np

import concourse.bass as bass
import concourse.bacc as bacc
import concourse.mybir as mybir
import concourse.tile as tile
from concourse import bass_utils
from concourse.alu_op_type import AluOpType

try:
    import ml_dtypes

    BF16_NP = ml_dtypes.bfloat16
except Exception:  # pragma: no cover
    BF16_NP = np.float32

F32 = mybir.dt.float32
BF16 = mybir.dt.bfloat16
AF = mybir.ActivationFunctionType

B, L, D = 16, 2048, 512
CF = 2048
TOPK = 22
KER = 25
EPS = 1e-5
SLOPE = 0.01
NCORES = 8
BPC = B // NCORES
DC = D // 128  # 4
CFC = CF // 128  # 16
TW = L // 512  # 4
TM = L // 128  # 16
NEG = -1.0e30

import os as _os_env


def _kp(name):
    return _os_env.environ.get(name, "1") == "1"


def build(nc: bass.Bass, n_group: int, lite: bool = False):
    x_dm = nc.dram_tensor("x_dm", [BPC, D, L], F32, kind="ExternalInput")
    xh_dm = nc.dram_tensor("xh_bf", [BPC, 128, DC, L], BF16, kind="ExternalInput")
    gt_d = nc.dram_tensor("gt_h", [128, DC, D], BF16, kind="ExternalInput")
    wvo_d = nc.dram_tensor("wvo_h", [128, DC, D], BF16, kind="ExternalInput")
    bop_d = nc.dram_tensor("bop_t", [128, DC], F32, kind="ExternalInput")
    w1_d = nc.dram_tensor("w1s", [CFC, 128, DC * 3, 128], BF16, kind="ExternalInput")
    w2_d = nc.dram_tensor("w2s", [DC, 128, 2, CFC * 3 // 2, 128], BF16,
                          kind="ExternalInput")
    lng_d = nc.dram_tensor("lng_t", [128, DC], F32, kind="ExternalInput")
    lnb_d = nc.dram_tensor("lnb_t", [128, DC], F32, kind="ExternalInput")
    out_dm = nc.dram_tensor("out_dm", [BPC, D, L], BF16, kind="ExternalOutput")

    with tile.TileContext(nc) as tc:
        _body(nc, tc, n_group, x_dm, xh_dm, gt_d, wvo_d, bop_d, w1_d, w2_d,
              lng_d, lnb_d, out_dm)
    return nc


def _decompose(nc, scan_pool, src, dst):
    for dci in range(DC):
        _decompose_dci(nc, scan_pool, src, dst, dci)


def _decompose_dci(nc, scan_pool, src, dst, dci):
    """dst[:, dci, 1:L+1] = src[:, dci] - movavg_KER; replicated edge cols."""
    half = (KER - 1) // 2
    if True:
        eng = nc.vector
        pad = scan_pool.tile([128, L + KER], F32, tag="scan_pad",
                             name="scan_pad")  # noqa
        cs = scan_pool.tile([128, L + KER], F32, tag="scan_cs",
                            name="scan_cs")
        eng.memset(pad[:, 0:1], 0.0)
        eng.tensor_copy(
            out=pad[:, 1 : 1 + half],
            in_=src[:, dci, 0:1].to_broadcast([128, half]),
        )
        nc.scalar.activation(pad[:, 1 + half : 1 + half + L], src[:, dci, :], AF.Copy)
        eng.tensor_copy(
            out=pad[:, 1 + half + L :],
            in_=src[:, dci, L - 1 : L].to_broadcast([128, half]),
        )
        eng.tensor_tensor_scan(
            out=cs[:], data0=pad[:], data1=pad[:], initial=0.0,
            op0=AluOpType.add, op1=AluOpType.bypass,
        )
        # d1 reuses pad (dead after the scan)
        eng.tensor_sub(out=pad[:, 0:L], in0=cs[:, KER:], in1=cs[:, 0:L])
        eng.scalar_tensor_tensor(
            out=dst[:, dci, 1 : L + 1], in0=pad[:, 0:L], scalar=-1.0 / KER,
            in1=src[:, dci, :], op0=AluOpType.mult, op1=AluOpType.add,
        )
        eng.tensor_copy(out=dst[:, dci, 0:1], in_=dst[:, dci, 1:2])
        eng.tensor_copy(
            out=dst[:, dci, L + 1 : L + 2], in_=dst[:, dci, L : L + 1]
        )


def _body(nc, tc, n_group, x_dm, xh_dm, gt_d, wvo_d, bop_d, w1_d, w2_d,
          lng_d, lnb_d, out_dm):
    with (
        tc.tile_pool(name="p0", bufs=1) as p0,
        tc.tile_pool(name="pp", bufs=4, space="PSUM") as pp,
        tc.tile_pool(name="dr", bufs=1, space="DRAM") as dr,
        tc.tile_pool(name="dr3", bufs=4, space="DRAM") as dr3,
        tc.tile_pool(name="pseab", bufs=2) as pseab,
        tc.tile_pool(name="pysb", bufs=1) as pysb,
        tc.tile_pool(name="psea2", bufs=1) as psea2,
        tc.tile_pool(name="pscan", bufs=1) as pscan,
    ):
        # ---------------- persistent constants ----------------
        ones_bf = p0.tile([128, 1], BF16, tag="ones_bf")
        nc.vector.memset(ones_bf[:], 1.0 / D)
        bop_c = p0.tile([128, DC], F32, tag="bop_c")
        lng_c = p0.tile([128, DC], F32, tag="lng_c")
        lnb_c = p0.tile([128, DC], F32, tag="lnb_c")
        nc.sync.dma_start(bop_c[:], bop_d[:, :])
        nc.sync.dma_start(lng_c[:], lng_d[:, :])
        nc.sync.dma_start(lnb_c[:], lnb_d[:, :])
        hb = []
        seab = []

        with tc.tile_pool(name="pxw", bufs=2) as pxw:
            xw = []
            gt_s = pxw.tile([128, DC, D], BF16, tag="gt_s")
            wvo_s = pxw.tile([128, DC, D], BF16, tag="wvo_s")
            nc.sync.dma_start(gt_s[:], gt_d[:, :, :])
            nc.sync.dma_start(wvo_s[:], wvo_d[:, :, :])
            pgbuf_ctx = tc.tile_pool(name="pgbuf", bufs=2)
            pgbuf = pgbuf_ctx.__enter__()
            gbufs = []
            pmv_ctx = tc.tile_pool(name="pmv", bufs=1)
            pmv = pmv_ctx.__enter__()
            mvf = pmv.tile([1, BPC * L], F32, tag="mvf")
            cco = []
            # ============ phase 1: mean_value + per-batch AllReduce ========
            with (
                tc.tile_pool(name="pxb", bufs=1) as pxb,
                tc.tile_pool(name="pxg", bufs=1) as pxg,
                tc.tile_pool(name="pwa", bufs=2) as pwa,
                tc.tile_pool(name="pcsb", bufs=2) as pcsb,
                tc.tile_pool(name="ppm1", bufs=1, space="PSUM") as ppm1,
            ):
                xbs = []
                for b in range(BPC):
                    xb = pxb.tile([128, DC, L], BF16, tag="xb")
                    if b == 0:
                        for w in range(TW):
                            nc.sync.dma_start(
                                xb[:, :, 512 * w : 512 * w + 512],
                                xh_dm.ap()[b, :, :, 512 * w : 512 * w + 512],
                            )
                    else:
                        nc.sync.dma_start(xb[:], xh_dm.ap()[b])
                    xbs.append(xb)

                    # xg = (Wq Wk^T) x  (d-major)
                    xg = pxg.tile([128, DC, L], BF16, tag="xg")
                    for dco in range(DC):
                        for twi in range(TW):
                            ps = pp.tile([128, 512], F32, tag="ps")
                            for dci in range(DC):
                                nc.tensor.matmul(
                                    ps[:],
                                    lhsT=gt_s[:, dci, 128 * dco : 128 * dco + 128],
                                    rhs=xb[:, dci, 512 * twi : 512 * twi + 512],
                                    start=(dci == 0), stop=(dci == DC - 1),
                                )
                            if (dco + twi) % 2 == 0:
                                nc.scalar.activation(
                                    xg[:, dco, 512 * twi : 512 * twi + 512],
                                    ps[:], AF.Copy,
                                )
                            else:
                                nc.vector.tensor_copy(
                                    out=xg[:, dco, 512 * twi : 512 * twi + 512],
                                    in_=ps[:],
                                )

                    # corr tiles + diagonal shear + ones-matmul lag reduction
                    mv_reg = [
                        ppm1.tile([1, 512], F32, tag=f"mv{cc}", name=f"mv_{cc}")
                        for cc in range(4)
                    ]

                    def _emit_mv(A, wa, mv_reg=mv_reg):
                        for cc in range(4):
                            w0 = (512 * cc + 128 * A) % L
                            nc.tensor.matmul(
                                mv_reg[cc][0:1, :],
                                lhsT=ones_bf[:],
                                rhs=wa[:, w0 : w0 + 512],
                                start=(A == 0), stop=(A == TM - 1),
                            )

                    def _xwproj(xb=xb):
                        xw_b = pxw.tile([128, TM, D], BF16, tag="xw",
                                        name="xw_b")
                        for tm in range(TM if _kp("KP3") else 0):
                            ps = pp.tile([128, 512], F32, tag="ps")
                            for dci in range(DC):
                                nc.tensor.matmul(
                                    ps[:],
                                    lhsT=xb[:, dci, 128 * tm : 128 * tm + 128],
                                    rhs=wvo_s[:, dci, :],
                                    start=(dci == 0), stop=(dci == DC - 1),
                                )
                            if tm % 2 == 0:
                                nc.scalar.activation(xw_b[:, tm, :], ps[:], AF.Copy)
                            else:
                                nc.vector.tensor_copy(out=xw_b[:, tm, :], in_=ps[:])
                        xw.append(xw_b)

                    pend = []
                    for A in range(TM if _kp("KP1") else 0):
                        if A == 8:
                            _xwproj()  # frees the xb slot before the next
                            # batch's load; fills PE during the shear tail
                        bufA = dr3.tile([128, 4224], BF16, tag="bufA")
                        for tB in range(TW):
                            psc = pp.tile([128, 512], F32, tag="ps")
                            for dci in range(DC):
                                nc.tensor.matmul(
                                    psc[:],
                                    lhsT=xb[:, dci, 128 * A : 128 * A + 128],
                                    rhs=xg[:, dci, 512 * tB : 512 * tB + 512],
                                    start=(dci == 0), stop=(dci == DC - 1),
                                )
                            c_sb = pcsb.tile([128, 512], BF16, tag="c_sb")
                            if tB % 2 == 0:
                                nc.scalar.activation(c_sb[:], psc[:], AF.Copy)
                            else:
                                nc.vector.tensor_copy(out=c_sb[:], in_=psc[:])
                            for cp, eng in ((0, nc.sync), (1, nc.scalar)):
                                dst = bass.AP(
                                    bufA[:].tensor,
                                    127 + 512 * tB + 2048 * cp,
                                    [[4223, 128], [1, 512]],
                                )
                                eng.dma_start(dst, c_sb[:])
                        wa = pwa.tile([128, 2560], BF16, tag="wa")
                        nc.sync.dma_start(
                            wa[:],
                            bass.AP(bufA[:].tensor, 128, [[4224, 128], [1, 2560]]),
                        )
                        pend.append((A, wa))
                        if len(pend) > 1:
                            _emit_mv(*pend.pop(0))
                    for a_w in pend:
                        _emit_mv(*a_w)
                    for cc in range(4):
                        nc.scalar.activation(
                            mvf[0:1, L * b + 512 * cc : L * b + 512 * cc + 512],
                            mv_reg[cc][0:1, :], AF.Copy,
                        )

                    # per-batch AllReduce, issued as soon as this mv is done
                    cci_b = dr.tile([1, L], F32, tag=f"cci{b}")
                    cco_b = dr.tile([1, L], F32, tag=f"cco{b}")
                    nc.gpsimd.dma_start(cci_b[:], mvf[0:1, L * b : L * b + L])
                    _selfcc = _os_env.environ.get("KERNEL_SELFCC", "0") == "1"
                    _nocc = _os_env.environ.get("KERNEL_NOCC", "0") == "1"
                    if _nocc:
                        nc.gpsimd.dma_start(cco_b[:], cci_b[:])
                    else:
                        nc.gpsimd.collective_compute(
                            "AllReduce", AluOpType.add,
                            replica_groups=(
                                [[c] for c in range(n_group)] if _selfcc
                                else [list(range(n_group))]
                            ),
                            ins=[cci_b[:].opt()], outs=[cco_b[:].opt()],
                        )
                    cco.append(cco_b)
                    if not _kp("KP1"):
                        _xwproj()

            # ---- phase 2: combine ARs, topk, masked softmax, band bufs
            with (
                tc.tile_pool(name="p12", bufs=1) as p12,
                tc.tile_pool(name="ppw", bufs=1, space="PSUM") as ppw,
            ):
                bsum = p12.tile([1, L], F32, tag="bsum")
                work = p12.tile([1, L], F32, tag="work")
                mask = p12.tile([1, L], F32, tag="mask")
                nbias = p12.tile([1, 1], F32, tag="nbias")
                nc.vector.memset(nbias[:], -1.0e4)
                pwm = ppw.tile([128, 512], F32, tag="pwm")

                def _warm(t):
                    # dummy matmul keyed on a just-written [1,*] tile: keeps
                    # the PE HAM window busy through the dependency-bound gap
                    nc.tensor.matmul(
                        pwm[:], lhsT=t[0:1, 0:128], rhs=t[0:1, 0:512],
                        start=True, stop=True,
                    )

                nc.gpsimd.dma_start(bsum[:], cco[0][:])
                nc.gpsimd.dma_start(work[:], cco[1][:])
                nc.vector.tensor_add(out=bsum[:], in0=bsum[:], in1=work[:])
                _warm(bsum)

                t_on = bsum
                for r, kk in enumerate((8, 8, TOPK - 16)):
                    mx8 = p12.tile([1, 8], F32, tag=f"mx8_{r}")
                    nc.vector.max(out=mx8[:], in_=t_on[:])
                    if kk < 8:
                        nc.vector.memset(mx8[:, kk:8], NEG)
                    nc.vector.match_replace(
                        out=work[:], in_to_replace=mx8[:], in_values=t_on[:],
                        imm_value=NEG,
                    )
                    t_on = work
                    _warm(work)
                nc.vector.tensor_sub(out=mask[:], in0=bsum[:], in1=work[:])
                nc.vector.tensor_scalar_min(mask[:], mask[:], 1.0)
                _warm(mask)

                for b in range(BPC):
                    # softmax over the 22 kept lags: gf = (mv+1e4)*mask,
                    # exp(gf-1e4) -> masked-out lanes underflow to exact 0
                    gf = bsum  # bsum is dead after the mask; reuse its slot
                    nc.vector.scalar_tensor_tensor(
                        out=gf[:], in0=mvf[0:1, L * b : L * b + L],
                        scalar=1.0e4, in1=mask[:],
                        op0=AluOpType.add, op1=AluOpType.mult,
                    )
                    nc.scalar.activation(gf[:], gf[:], AF.Exp, bias=nbias[0:1, 0:1])
                    _warm(gf)
                    zz = p12.tile([1, 1], F32, tag="sm_z")
                    nc.vector.reduce_sum(
                        out=zz[:], in_=gf[:], axis=mybir.AxisListType.X
                    )
                    nc.vector.reciprocal(out=zz[:], in_=zz[:])
                    gfb = p12.tile([1, L], BF16, tag=f"gfb{b}")
                    nc.vector.tensor_scalar_mul(gfb[:], gf[:], zz[:])
                    # periodic replication into DRAM; a row-step-2047 read
                    # yields the circulant band gbuf[p,m] = g[(127-p+m)%L].
                    # b0 on the sync ring, b1 on scalar -> the two chains
                    # don't head-of-line block each other.
                    deng = nc.sync if b == 0 else nc.scalar
                    hbuf = dr.tile([1, 129 * L], BF16, tag=f"hb{b}")
                    _gs = gfb[:]
                    _ga = [list(p) for p in _gs.ap]
                    grep_ap = bass.AP(
                        _gs.tensor, _gs.offset, [_ga[0], [0, 129], _ga[-1]]
                    )
                    deng.dma_start(
                        hbuf[:].rearrange("a (r n) -> a r n", r=129), grep_ap
                    )
                    hb.append(hbuf)
                    gbuf = pgbuf.tile([128, 3968], BF16, tag="gbuf",
                                      name="gbuf")
                    deng.dma_start(
                        gbuf[:],
                        bass.AP(hbuf[:].tensor, 127, [[2047, 128], [1, 3968]]),
                    )
                    gbufs.append(gbuf)

            pmv_ctx.__exit__(None, None, None)
            # ============ phase 3+4 per batch: circulant + decomp ==========
            with (
                tc.tile_pool(name="pacx", bufs=2) as pacx,
                tc.tile_pool(name="pxr", bufs=3) as pxr,
            ):
                for b in range(BPC):
                    gbuf = gbufs[b]
                    acx = pacx.tile([128, DC, L], BF16, tag="acx")
                    for dm in range(DC if _kp("KP3") else 0):
                        for nw in range(TW):
                            ps = pp.tile([128, 512], F32, tag="ps")
                            for Bc in range(TM):
                                gp = 512 * nw - 128 * Bc + 1920
                                nc.tensor.matmul(
                                    ps[:],
                                    lhsT=xw[b][:, Bc, 128 * dm : 128 * dm + 128],
                                    rhs=gbuf[:, gp : gp + 512],
                                    start=(Bc == 0), stop=(Bc == TM - 1),
                                )
                            xr = pxr.tile([128, 512], F32, tag="xr")
                            nc.sync.dma_start(
                                xr[:],
                                x_dm.ap()[b, 128 * dm : 128 * dm + 128,
                                          512 * nw : 512 * nw + 512],
                            )
                            nc.vector.scalar_tensor_tensor(
                                out=acx[:, dm, 512 * nw : 512 * nw + 512],
                                in0=ps[:], scalar=bop_c[:, dm : dm + 1], in1=xr[:],
                                op0=AluOpType.add, op1=AluOpType.add,
                            )
                    sb = pseab.tile([128, DC, L + 2], BF16, tag="seab")
                    if _kp("KP4"):
                        _decompose(nc, pscan, acx, sb)
                    seab.append(sb)

            pgbuf_ctx.__exit__(None, None, None)

        # ============ conv + decomp2 + layernorm, interleaved =============
        with (
            tc.tile_pool(name="pw1", bufs=4) as pw1,
            tc.tile_pool(name="pw2", bufs=2) as pw2,
            tc.tile_pool(name="ph1r", bufs=1) as ph1r,
            tc.tile_pool(name="pc5", bufs=2) as pc5,
            tc.tile_pool(name="pln", bufs=1) as pln,
            tc.tile_pool(name="pog", bufs=2) as pog,
            tc.tile_pool(name="ppm7", bufs=2, space="PSUM") as ppm7,
        ):
            # h1 ring: 2 self-contained window slots [left halo | 512 | right]
            h1s = [
                ph1r.tile([128, CFC, 514], BF16, tag=f"h1s{s}", name=f"h1s_{s}")
                for s in range(2)
            ]

            def conv1_win(b, nw):
                slot = h1s[nw % 2]
                sb = seab[b]
                for co in range(CFC if _kp("KP5") else 0):
                    w1t = pw1.tile([128, DC * 3, 128], BF16, tag="w1t")
                    nc.sync.dma_start(w1t[:], w1_d.ap()[co])
                    ps = pp.tile([128, 512], F32, tag="ps")
                    first = True
                    for dci in range(DC):
                        for tap in range(3):
                            nc.tensor.matmul(
                                ps[:],
                                lhsT=w1t[:, 3 * dci + tap, :],
                                rhs=sb[:, dci,
                                       512 * nw + tap : 512 * nw + tap + 512],
                                start=first, stop=(dci == DC - 1 and tap == 2),
                            )
                            first = False
                    nc.scalar.activation(
                        slot[:, co, 1:513], ps[:], AF.Lrelu, alpha=SLOPE
                    )
                # halo columns
                if nw == 0:
                    nc.vector.tensor_copy(out=slot[:, :, 0:1], in_=slot[:, :, 1:2])
                else:
                    nc.vector.tensor_copy(
                        out=slot[:, :, 0:1], in_=h1s[(nw - 1) % 2][:, :, 512:513]
                    )
                    nc.vector.tensor_copy(
                        out=h1s[(nw - 1) % 2][:, :, 513:514], in_=slot[:, :, 1:2]
                    )
                if nw == TW - 1:
                    nc.vector.tensor_copy(
                        out=slot[:, :, 513:514], in_=slot[:, :, 512:513]
                    )

            def conv2_win(b, nw, ysb, post_co=None):
                slot = h1s[nw % 2]
                for co in range(DC if _kp("KP6") else 0):
                    ps = pp.tile([128, 512], F32, tag="ps")
                    first = True
                    for hw in range(2):
                        w2t = pw2.tile([128, CFC * 3 // 2, 128], BF16, tag="w2t")
                        nc.scalar.dma_start(w2t[:], w2_d.ap()[co, :, hw])
                        for k in range(CFC * 3 // 2):
                            ci, tap = divmod(hw * CFC * 3 // 2 + k, 3)
                            nc.tensor.matmul(
                                ps[:],
                                lhsT=w2t[:, k, :],
                                rhs=slot[:, ci, tap : tap + 512],
                                start=first,
                                stop=(hw == 1 and k == CFC * 3 // 2 - 1),
                            )
                            first = False
                    h2r = pc5.tile([128, 512], F32, tag="h2r")
                    nc.scalar.activation(h2r[:], ps[:], AF.Lrelu, alpha=SLOPE)
                    nc.vector.tensor_add(
                        out=ysb[:, co, 512 * nw : 512 * nw + 512],
                        in0=h2r[:],
                        in1=seab[b][:, co, 1 + 512 * nw : 513 + 512 * nw],
                    )
                    if post_co is not None:
                        post_co(co)

            def conv_batch(b, ysb, post_co=None):
                conv1_win(b, 0)
                conv1_win(b, 1)
                conv2_win(b, 0, ysb)
                conv1_win(b, 2)
                conv2_win(b, 1, ysb)
                conv1_win(b, 3)
                conv2_win(b, 2, ysb)
                conv2_win(b, 3, ysb, post_co=post_co)

            def phase7(b, ysb, sea2):
                if _kp("KP7") and b == 0:
                    _decompose(nc, pscan, ysb, sea2)
                stats = pln.tile([1, 2 * L], F32, tag="stats")
                for twi in range(TW if _kp("KP7") else 0):
                    st_s = ppm7.tile([1, 512], F32, tag="st_s")
                    st_q = ppm7.tile([1, 512], F32, tag="st_q")
                    for dci in range(DC):
                        sqt = pc5.tile([128, 512], BF16, tag="sqt")
                        nc.scalar.activation(
                            sqt[:],
                            sea2[:, dci, 1 + 512 * twi : 513 + 512 * twi],
                            AF.Square,
                        )
                        nc.tensor.matmul(
                            st_s[0:1, :], lhsT=ones_bf[:],
                            rhs=sea2[:, dci, 1 + 512 * twi : 513 + 512 * twi],
                            start=(dci == 0), stop=(dci == DC - 1),
                        )
                        nc.tensor.matmul(
                            st_q[0:1, :], lhsT=ones_bf[:], rhs=sqt[:],
                            start=(dci == 0), stop=(dci == DC - 1),
                        )
                    nc.scalar.activation(
                        stats[0:1, 512 * twi : 512 * twi + 512],
                        st_s[0:1, :], AF.Copy,
                    )
                    nc.scalar.activation(
                        stats[0:1, L + 512 * twi : L + 512 * twi + 512],
                        st_q[0:1, :], AF.Copy,
                    )
                if _kp("KP7"):
                    # fold stats [1,2L] -> [128,16]x2 via DRAM for fast rsqrt
                    st_d = dr.tile([1, 2 * L], F32, tag=f"st_d{b}")
                    nc.sync.dma_start(st_d[:], stats[:])
                    muf = pln.tile([128, 16], F32, tag="muf")
                    msf = pln.tile([128, 16], F32, tag="msf")
                    nc.sync.dma_start(
                        muf[:], bass.AP(st_d[:].tensor, 0, [[16, 128], [1, 16]])
                    )
                    nc.sync.dma_start(
                        msf[:], bass.AP(st_d[:].tensor, L, [[16, 128], [1, 16]])
                    )
                    varf = pln.tile([128, 16], F32, tag="varf")
                    nc.vector.tensor_mul(out=varf[:], in0=muf[:], in1=muf[:])
                    nc.vector.tensor_sub(out=varf[:], in0=msf[:], in1=varf[:])
                    nc.vector.tensor_scalar_add(varf[:], varf[:], EPS)
                    nc.vector.reciprocal(out=varf[:], in_=varf[:])
                    nc.scalar.activation(varf[:], varf[:], AF.Sqrt)
                    rs_d = dr.tile([1, L], F32, tag=f"rs_d{b}")
                    nc.sync.dma_start(
                        bass.AP(rs_d[:].tensor, 0, [[16, 128], [1, 16]]), varf[:]
                    )
                    mub = pln.tile([128, L], F32, tag="mub")
                    rsb = pln.tile([128, L], F32, tag="rsb")
                    nc.sync.dma_start(
                        mub[:], bass.AP(st_d[:].tensor, 0, [[0, 128], [1, L]])
                    )
                    nc.sync.dma_start(
                        rsb[:], bass.AP(rs_d[:].tensor, 0, [[0, 128], [1, L]])
                    )
                for dci in range(DC if _kp("KP7") else 0):
                    eng = nc.vector
                    og = pog.tile([128, L], BF16, tag="og")
                    eng.tensor_sub(
                        out=og[:], in0=sea2[:, dci, 1 : L + 1], in1=mub[:]
                    )
                    eng.tensor_mul(out=og[:], in0=og[:], in1=rsb[:])
                    nc.scalar.activation(
                        og[:], og[:], AF.Identity,
                        bias=lnb_c[:, dci : dci + 1], scale=lng_c[:, dci : dci + 1],
                    )
                    nc.scalar.dma_start(
                        out_dm.ap()[b, 128 * dci : 128 * dci + 128, :], og[:]
                    )

            ysb0 = pysb.tile([128, DC, L], BF16, tag="ysb")
            sea2_0 = psea2.tile([128, DC, L + 2], BF16, tag="sea2",
                                name="sea2_0")
            conv_batch(0, ysb0)
            phase7(0, ysb0, sea2_0)
            ysb1 = pysb.tile([128, DC, L], BF16, tag="ysb")
            sea2_1 = psea2.tile([128, DC, L + 2], BF16, tag="sea2",
                                name="sea2_1")
            conv_batch(1, ysb1, post_co=(
                (lambda co: _decompose_dci(nc, pscan, ysb1, sea2_1, co))
                if _kp("KP7") else None))
            phase7(1, ysb1, sea2_1)


# ---------------------------------------------------------------------------
# host side
# ---------------------------------------------------------------------------
_CACHE = {}


def _get_nc(n_group: int):
    if n_group not in _CACHE:
        nc = bacc.Bacc("TRN2", target_bir_lowering=False, debug=False,
                       num_devices=n_group)
        build(nc, n_group)
        nc.compile()
        _CACHE[n_group] = nc
    return _CACHE[n_group]


def stage_inputs(inputs, ncores=NCORES):
    x = np.asarray(inputs["x"], np.float32)
    Wq = np.asarray(inputs["Wq"], np.float32)
    Wk = np.asarray(inputs["Wk"], np.float32)
    Wv = np.asarray(inputs["Wv"], np.float32)
    Wo = np.asarray(inputs["Wo"], np.float32)
    bv = np.asarray(inputs["bv"], np.float32)
    bo = np.asarray(inputs["bo"], np.float32)
    w1 = np.asarray(inputs["conv1_w"], np.float32)
    w2 = np.asarray(inputs["conv2_w"], np.float32)
    lng = np.asarray(inputs["ln_g"], np.float32)
    lnb = np.asarray(inputs["ln_b"], np.float32)

    bop = bo + bv @ Wo
    col = lambda v: np.ascontiguousarray(v.reshape(DC, 128).T)
    dmaj = lambda M: np.ascontiguousarray(
        M.reshape(DC, 128, D).transpose(1, 0, 2)
    ).astype(BF16_NP)
    # corr = x^T (Wq Wk^T) x  ->  xg = (Wq Wk^T) x, staged pre-transposed
    gt_h = dmaj(Wk @ Wq.T)
    # rolls commute with channel mixing: fold Wv@Wo
    wvo_h = dmaj(Wv @ Wo)
    w1s = np.ascontiguousarray(
        w1.reshape(3, DC, 128, CFC, 128).transpose(3, 2, 1, 0, 4)
    ).reshape(CFC, 128, DC * 3, 128).astype(BF16_NP)
    # w2 staged as [co, p, 48, 128] with the 48 (ci,tap) pairs in order,
    # then split into two halves of 24 for streaming
    w2s = np.ascontiguousarray(
        w2.reshape(3, CFC, 128, DC, 128).transpose(3, 2, 1, 0, 4)
    ).reshape(DC, 128, 2, CFC * 3 // 2, 128).astype(BF16_NP)

    shared = {
        "gt_h": gt_h, "wvo_h": wvo_h, "bop_t": col(bop),
        "w1s": w1s, "w2s": w2s, "lng_t": col(lng), "lnb_t": col(lnb),
    }
    bpc = B // ncores
    in_maps = []
    for c in range(ncores):
        m = dict(shared)
        xc = np.ascontiguousarray(x[bpc * c : bpc * (c + 1)].transpose(0, 2, 1))
        m["x_dm"] = xc
        m["xh_bf"] = np.ascontiguousarray(
            xc.reshape(bpc, DC, 128, L).transpose(0, 2, 1, 3)
        ).astype(BF16_NP)
        in_maps.append(m)
    return in_maps


def kernel(**inputs):
    nc = _get_nc(NCORES)
    in_maps = stage_inputs(inputs)
    res = bass_utils.run_bass_kernel_spmd(nc, in_maps, core_ids=list(range(NCORES)))
    out = np.empty((B, L, D), np.float32)
    for c in range(NCORES):
        o = np.asarray(res.results[c]["out_dm"])  # [BPC, D, L] bf16
        for i in range(BPC):
            out[BPC * c + i] = o[i].T.astype(np.float32)
    return out
